# revision 1
# baseline (speedup 1.0000x reference)
"""Causal self-attention (GQA + RoPE) Trainium2 kernel, 8-way sharded.

Sharding: DP=4 over batch x TP=2 over kv-head groups (2 kv heads + their
8 q heads per group).  Each core computes its batch's qkv projection for
its head group, causal attention, and a partial c_proj (columns of
w_proj for its head group).  Host sums the two partial c_proj outputs
per batch.

Everything on-chip runs transposed ([feature, token] layout) so matmuls
contract along partitions; host transposes inputs/outputs.

Pipeline: the attention inner loop is ACT-bound (one exp per QK tile),
so the q/k projection + RoPE work for head h+1 is interleaved into the
PE stream of head h's attention, keeping the PE busy while ACT churns
through exps.

RoPE: w_attn q/k rows are permuted per-head to [even dims; odd dims] so
rotation pairs land at partition f and f+64 of the qkv psum tile:
  P  = ps * [c; c] (SBUF),  P2 = ps * [s; s] (PSUM)
  out[0:64]   = P[0:64]  - P2[64:128]
  out[64:128] = P2[0:64] + P[64:128]
(each combine reads one SBUF + one PSUM operand, which may sit at
different base partitions; two SBUF operands may not).

Softmax: att^T tiles ([k, q] layout) are exp'd on ACT without
max-subtraction (logits are O(6), fp32-safe).  Denominators: groups of
4 e-tiles are tree-summed on DVE and hit with one ones-column matmul
per group (deferred into the next group's PE stream); the per-q
reciprocal is broadcast down partitions with a f32r outer-product
matmul, also deferred one q-tile.
"""

import math

import numpy as np
import ml_dtypes

import concourse.bass as bass
import concourse.mybir as mybir
import concourse.tile as tile
from concourse import bacc
from concourse.bass_utils import run_bass_kernel_spmd

ALU = mybir.AluOpType
AF = mybir.ActivationFunctionType
F32 = mybir.dt.float32
F32R = mybir.dt.float32r
BF16 = mybir.dt.bfloat16
BF = ml_dtypes.bfloat16

# problem shape (hardcoded per contest rules)
B, T, C = 4, 2048, 2048
N_HEAD, N_KV_HEAD, HD = 16, 4, 128
ROPE_THETA = 10000.0

TP = 2            # head-group shards
DP = 4            # batch shards
HQ = N_HEAD // TP         # 8 q heads per core
HKV = N_KV_HEAD // TP     # 2 kv heads per core
NREP = N_HEAD // N_KV_HEAD  # 4
QK_ROWS = (HQ + HKV) * HD   # 1280
KC = C // 128     # 16 contraction tiles
NQ = T // 512     # 4 token strips
MQK = QK_ROWS // 128  # 10 feature tiles (8 q heads + 2 kv heads)
FM = C // 128     # 16 output feature tiles
SCALE = 1.0 / math.sqrt(HD)

N_CORES = 8

_NC = None        # cached compiled Bass module
LAST_RUN = None   # BassKernelResults of the most recent kernel() call


def build_nc(dbg=False):
    nc = bacc.Bacc(None, target_bir_lowering=False, debug=False)

    xT = nc.declare_dram_parameter("xT", [C, T], BF16, isOutput=False)
    wqk3 = nc.declare_dram_parameter("wqk3", [MQK, 128, C], BF16, isOutput=False)
    wv3 = nc.declare_dram_parameter("wv3", [128, KC * HKV * HD], BF16, isOutput=False)
    wp5 = nc.declare_dram_parameter("wp5", [FM, 128, HQ, 128], BF16, isOutput=False)
    trigf = nc.declare_dram_parameter("trigf", [128, T], F32, isOutput=False)  # [c;c]
    trigw = nc.declare_dram_parameter("trigw", [128, T], F32, isOutput=False)  # [s;s]
    maskd = nc.declare_dram_parameter("maskd", [4, 128, 512], BF16, isOutput=False)
    outT = nc.declare_dram_parameter("outT", [C, T], F32, isOutput=True)
    if dbg:
        dbg_q = nc.declare_dram_parameter("dbg_q", [128, T], BF16, isOutput=True)
        dbg_k = nc.declare_dram_parameter("dbg_k", [128, T], BF16, isOutput=True)
        dbg_v = nc.declare_dram_parameter(
            "dbg_v", [128, T // 128, HKV * HD], BF16, isOutput=True
        )
        dbg_y = nc.declare_dram_parameter("dbg_y", [128, HQ, T], BF16, isOutput=True)

    with tile.TileContext(nc) as tc:
        with (
            tc.tile_pool(name="const", bufs=1) as const,
            tc.tile_pool(name="persist", bufs=1) as persist,
            tc.tile_pool(name="eb", bufs=6) as eb,
            tc.tile_pool(name="gag", bufs=2) as gag,
            tc.tile_pool(name="rb", bufs=1) as rb,
            tc.tile_pool(name="psS", bufs=2, space="PSUM") as psS,
            tc.tile_pool(name="psY", bufs=2, space="PSUM") as psY,
            tc.tile_pool(name="psD", bufs=2, space="PSUM") as psD,
        ):
            trigf_sb = const.tile([128, T], F32, name="trigf")
            trigw_sb = const.tile([128, T], F32, name="trigw")
            mask_sb = const.tile([128, 4, 512], BF16, name="mask")
            ones_col = const.tile([128, 1], BF16, name="onec")
            ones_row_f = const.tile([1, 128], F32, name="onerf")
            ones_row = const.tile([1, 128], F32R, name="oner")

            qrot = [persist.tile([128, T], BF16, name=f"qrot{h}") for h in range(HQ)]
            krot = [persist.tile([128, T], BF16, name=f"krot{h}") for h in range(HKV)]
            v_sb = persist.tile([128, T // 128, HKV * HD], BF16, name="vtok")
            yt = persist.tile([128, HQ, T], BF16, name="yt")

            state = {"pending": None, "pending_ones": None}

            def finalize(h, qj, ps_y, ps_d):
                rec_f = rb.tile([1, 512], F32, name="recf")
                rec_r = rb.tile([1, 512], F32R, name="recr")
                r_sb = rb.tile([128, 512], F32, name="r")
                nc.vector.reciprocal(rec_f[:], ps_d[:])
                with nc.allow_low_precision("f32r recip broadcast"):
                    nc.vector.tensor_copy(rec_r[:], rec_f[:])
                ps_r = psS.tile([128, 512], F32, name="pss")
                nc.tensor.matmul(
                    ps_r[:], ones_row[:], rec_r[:], start=True, stop=True
                )
                nc.scalar.copy(r_sb[:], ps_r[:])
                nc.vector.tensor_tensor(
                    yt[:, h, bass.ts(qj, 512)], ps_y[:], r_sb[:], ALU.mult
                )

            def flush_ones():
                if state["pending_ones"] is not None:
                    po, st, sp, pd = state["pending_ones"]
                    nc.tensor.matmul(pd[:], ones_col[:], po[:], start=st, stop=sp)
                    state["pending_ones"] = None

            def emit_qj(h, qj, pop):
                """Attention for (h, qj): QK tiles, exp, mask, AV, denominators.

                `pop()` is called once per k-tile to interleave filler PE work.
                Returns the (ps_y, ps_d) accumulators (not yet finalized).
                """
                kvh = h // NREP
                qsl = bass.ts(qj, 512)
                ps_y = psY.tile([128, 512], F32, name="psy")
                ps_d = psD.tile([1, 512], F32, name="psd")
                nkt = 4 * qj + 4
                g0 = ga = g2 = None
                for kt in range(nkt):
                    d = kt - 4 * qj
                    # diagonal tile d has valid q-columns only in [128d, 512)
                    lo = 128 * d if d > 0 else 0
                    qlo = qj * 512 + lo
                    ps_s = psS.tile([128, 512], F32, name="pss")
                    nc.tensor.matmul(
                        ps_s[:, lo:512],
                        krot[kvh][:, kt * 128 : (kt + 1) * 128],
                        qrot[h][:, qlo : (qj + 1) * 512],
                        start=True,
                        stop=True,
                    )
                    e = eb.tile([128, 512], BF16, name="e")
                    nc.scalar.activation(
                        e[:, lo:512], ps_s[:, lo:512], AF.Exp, scale=SCALE
                    )
                    if d >= 0:
                        nc.vector.tensor_tensor(
                            e[:, lo:512], e[:, lo:512],
                            mask_sb[:, d, lo:512], ALU.mult,
                        )
                    nc.tensor.matmul(
                        ps_y[:, lo:512],
                        v_sb[:, kt, kvh * HD : (kvh + 1) * HD],
                        e[:, lo:512],
                        start=(kt == 0),
                        stop=(kt == nkt - 1),
                    )
                    if d >= 0:
                        # diagonal group: narrow per-tile ones-matmuls
                        if d == 0:
                            flush_ones()
                        nc.tensor.matmul(
                            ps_d[:, lo:512],
                            ones_col[:],
                            e[:, lo:512],
                            start=(qj == 0 and kt == 0),
                            stop=(kt == nkt - 1),
                        )
                    else:
                        # full groups: tree-sum 4 e-tiles on DVE, one deferred
                        # ones-matmul per group (emitted in a later PE slot so
                        # the PE never waits on the DVE adds).
                        ph = kt % 4
                        if ph == 0:
                            g0 = e
                        elif ph == 1:
                            ga = gag.tile([128, 512], BF16, name="ga")
                            nc.vector.tensor_tensor(ga[:], g0[:], e[:], ALU.add)
                        elif ph == 2:
                            g2 = e
                        else:
                            gs = gag.tile([128, 512], BF16, name="gs")
                            nc.vector.tensor_tensor(gs[:], g2[:], e[:], ALU.add)
                            nc.vector.tensor_tensor(gs[:], gs[:], ga[:], ALU.add)
                            flush_ones()
                            grp = kt // 4
                            state["pending_ones"] = (gs, grp == 0, False, ps_d)
                    pop(kt)
                return ps_y, ps_d

            # ======== projection machinery (lives through heads 0..6) ========
            with (
                tc.tile_pool(name="xa", bufs=1) as xa,
                tc.tile_pool(name="wm", bufs=3) as wm,
                tc.tile_pool(name="ta", bufs=1) as ta,
                tc.tile_pool(name="psA", bufs=1, space="PSUM") as psA,
                tc.tile_pool(name="psP2", bufs=1, space="PSUM") as psP2,
            ):
                xs = xa.tile([128, KC, T], BF16, name="xs")

                def load_wm(m):
                    w = wm.tile([128, KC, 128], BF16, name="wm")
                    wsrc = wqk3[m, :, :].rearrange("p (kc c) -> p kc c", kc=KC)
                    for i in range(4):
                        nc.sync.dma_start(
                            w[:, 4 * i : 4 * i + 4, :], wsrc[:, 4 * i : 4 * i + 4, :]
                        )
                    return w

                def rope_ops(m, n, ps):
                    """The four RoPE ops for one (feature tile, strip) pair."""
                    dst = qrot[m] if m < HQ else krot[m - HQ]
                    nsl = bass.ts(n, 512)
                    pt = ta.tile([128, 512], F32, name="pt")
                    p2 = psP2.tile([128, 512], F32, name="p2")
                    yield nc.vector.tensor_tensor(
                        pt[:], ps[:], trigf_sb[:, nsl], ALU.mult
                    )
                    yield nc.vector.tensor_tensor(
                        p2[:], ps[:], trigw_sb[:, nsl], ALU.mult
                    )
                    yield nc.vector.tensor_tensor(
                        dst[0:64, nsl], pt[0:64, :], p2[64:128, :], ALU.subtract
                    )
                    yield nc.vector.tensor_tensor(
                        dst[64:128, nsl], p2[0:64, :], pt[64:128, :], ALU.add
                    )

                def a_stream(m, pool):
                    w = load_wm(m)
                    yield
                    for n in range(NQ):
                        nsl = bass.ts(n, 512)
                        ps = pool.tile([128, 512], F32, name="psA")
                        for kc in range(KC):
                            nc.tensor.matmul(
                                ps[:],
                                w[:, kc, :],
                                xs[:, kc, nsl],
                                start=(kc == 0),
                                stop=(kc == KC - 1),
                            )
                            if kc % 2 == 1:
                                yield
                        for _ in rope_ops(m, n, ps):
                            yield

                # ---- A0: v projection + k heads + q head 0 (pure PE phase) ----
                with tc.tile_pool(name="wvp", bufs=1) as wvp:
                    wv_sb = wvp.tile([128, KC, HKV * HD], BF16, name="wv")
                    wvsrc = wv3.rearrange("p (kc c) -> p kc c", kc=KC)
                    for i in range(4):
                        nc.sync.dma_start(
                            wv_sb[:, 4 * i : 4 * i + 4, :],
                            wvsrc[:, 4 * i : 4 * i + 4, :],
                        )
                    for kc in range(KC):
                        nc.sync.dma_start(
                            xs[:, kc, bass.ts(0, 512)],
                            xT[kc * 128 : (kc + 1) * 128, bass.ts(0, 512)],
                        )
                    nc.sync.dma_start(trigf_sb[:], trigf[:])
                    nc.sync.dma_start(trigw_sb[:], trigw[:])
                    nc.sync.dma_start(mask_sb[:], maskd.rearrange("d p q -> p d q"))
                    nc.vector.memset(ones_col[:], 1.0)
                    nc.vector.memset(ones_row_f[:], 1.0)
                    with nc.allow_low_precision("f32r ones for recip broadcast"):
                        nc.vector.tensor_copy(ones_row[:], ones_row_f[:])
                    wk0 = load_wm(HQ)
                    wk1 = load_wm(HQ + 1)
                    wq0 = load_wm(0)
                    for n in range(NQ):
                        nsl = bass.ts(n, 512)
                        if n + 1 < NQ:
                            nsl_next = bass.ts(n + 1, 512)
                            for kc in range(KC):
                                nc.sync.dma_start(
                                    xs[:, kc, nsl_next],
                                    xT[kc * 128 : (kc + 1) * 128, nsl_next],
                                )
                        for tt in range(4 * n, 4 * n + 4):
                            # reuse the attention-phase psum slots during A0
                            psv = psS.tile([128, 512], F32, name="pss")[
                                :, : HKV * HD
                            ]
                            for kc in range(KC):
                                nc.tensor.matmul(
                                    psv[:],
                                    xs[:, kc, tt * 128 : (tt + 1) * 128],
                                    wv_sb[:, kc, :],
                                    start=(kc == 0),
                                    stop=(kc == KC - 1),
                                )
                            nc.scalar.copy(v_sb[:, tt, :], psv[:])
                        for m, w in ((HQ, wk0), (HQ + 1, wk1), (0, wq0)):
                            ps = psY.tile([128, 512], F32, name="psy")
                            for kc in range(KC):
                                nc.tensor.matmul(
                                    ps[:],
                                    w[:, kc, :],
                                    xs[:, kc, nsl],
                                    start=(kc == 0),
                                    stop=(kc == KC - 1),
                                )
                            for _ in rope_ops(m, n, ps):
                                pass

                # ---- heads 0..6: attention + next head's projection ----
                for h in range(HQ - 1):
                    agen = a_stream(h + 1, psA)

                    def pop(kt, agen=agen):
                        next(agen, None)
                        if kt < 5 or kt >= 10:
                            next(agen, None)

                    for qj in range(NQ):
                        ps_y, ps_d = emit_qj(h, qj, pop)
                        if state["pending"] is not None:
                            finalize(*state["pending"])
                        state["pending"] = (h, qj, ps_y, ps_d)
                    for _ in agen:
                        pass

            # ---- head 7: attention + output projection interleaved ----
            with (
                tc.tile_pool(name="wpc", bufs=3) as wpc,
                tc.tile_pool(name="obp", bufs=3) as obp,
                tc.tile_pool(name="psO", bufs=2, space="PSUM") as psO,
            ):
                def c_stream(n):
                    """Output projection for token strip n (16 feature tiles)."""
                    nsl = bass.ts(n, 512)
                    for fm in range(FM):
                        wmc = wpc.tile([128, HQ, 128], BF16, name="wpc")
                        nc.sync.dma_start(wmc[:], wp5[fm, :, :, :])
                        yield
                        ps_o = psO.tile([128, 512], F32, name="pso")
                        for h2 in range(HQ):
                            nc.tensor.matmul(
                                ps_o[:],
                                wmc[:, h2, :],
                                yt[:, h2, nsl],
                                start=(h2 == 0),
                                stop=(h2 == HQ - 1),
                            )
                            if h2 % 2 == 1:
                                yield
                        ob = obp.tile([128, 512], F32, name="ob")
                        nc.scalar.copy(ob[:], ps_o[:])
                        nc.sync.dma_start(
                            outT[fm * 128 : (fm + 1) * 128, nsl], ob[:]
                        )
                        yield

                cgens = []

                _end = object()

                def pop7(kt):
                    for _ in range(2):
                        while cgens:
                            if next(cgens[0], _end) is _end:
                                cgens.pop(0)
                                continue
                            break

                for qj in range(NQ):
                    ps_y, ps_d = emit_qj(HQ - 1, qj, pop7)
                    flush_ones()
                    if state["pending"] is not None:
                        finalize(*state["pending"])
                        state["pending"] = None
                    finalize(HQ - 1, qj, ps_y, ps_d)
                    cgens.append(c_stream(qj))
                # drain remaining output projection
                for g in cgens:
                    for _ in g:
                        pass

            if dbg:
                nc.sync.dma_start(dbg_q[:], qrot[0][:])
                nc.sync.dma_start(dbg_k[:], krot[0][:])
                nc.sync.dma_start(dbg_v[:], v_sb[:])
                nc.sync.dma_start(dbg_y[:], yt[:])

    nc.compile()
    return nc


def _get_nc():
    global _NC
    if _NC is None:
        _NC = build_nc()
    return _NC


def _prep_inputs(x, w_attn, w_proj):
    """Build the 8 per-core input maps from the full-problem arrays."""
    perm = np.concatenate([np.arange(0, HD, 2), np.arange(1, HD, 2)])

    f = np.arange(64, dtype=np.float64)
    inv = ROPE_THETA ** (-2.0 * f / HD)
    ang = inv[:, None] * np.arange(T, dtype=np.float64)[None, :]
    trigc = np.cos(ang).astype(np.float32)
    trigs = np.sin(ang).astype(np.float32)
    trigf = np.ascontiguousarray(np.concatenate([trigc, trigc], axis=0))
    trigw = np.ascontiguousarray(np.concatenate([trigs, trigs], axis=0))

    kk = np.arange(128)[None, :, None]
    qq = np.arange(512)[None, None, :]
    dd = np.arange(4)[:, None, None]
    maskd = ((128 * dd + kk) <= qq).astype(BF)

    w_attn = np.asarray(w_attn)
    w_proj = np.asarray(w_proj)
    x = np.asarray(x)

    in_maps = []
    for core in range(N_CORES):
        b, g = core // TP, core % TP
        xTa = np.ascontiguousarray(x[b].T).astype(BF)

        qrows = []
        for h in range(HQ):
            gh = g * HQ + h
            qrows.append(gh * HD + perm)
        for kv in range(HKV):
            gk = g * HKV + kv
            qrows.append(N_HEAD * HD + gk * HD + perm)
        qrows = np.concatenate(qrows)
        wqk = w_attn[qrows].astype(BF)  # [1280, C]
        # wqk3[m, p, kc*128+col] = wqk[m*128+col, kc*128+p]
        wqk3 = np.ascontiguousarray(
            wqk.reshape(MQK, 128, KC, 128).transpose(0, 3, 2, 1).reshape(MQK, 128, C)
        )

        vrows = np.concatenate(
            [
                (N_HEAD + N_KV_HEAD) * HD + (g * HKV + kv) * HD + np.arange(HD)
                for kv in range(HKV)
            ]
        )
        wv = w_attn[vrows].astype(BF)  # [256, C]
        # wv3[p, kc*256+c] = wv[c, kc*128+p]
        wv3 = np.ascontiguousarray(
            wv.reshape(HKV * HD, KC, 128).transpose(2, 1, 0).reshape(128, KC * HKV * HD)
        )

        cols = np.arange(g * HQ * HD, (g + 1) * HQ * HD)
        wpg = w_proj[:, cols].astype(BF)  # [C, 1024], rows = out features
        # wp5[fm, d, h, p] = wpg[fm*128+p, h*128+d]
        wp5 = np.ascontiguousarray(
            wpg.T.reshape(HQ, 128, FM, 128).transpose(2, 1, 0, 3)
        )

        in_maps.append(
            {
                "xT": xTa,
                "wqk3": wqk3,
                "wv3": wv3,
                "wp5": wp5,
                "trigf": trigf,
                "trigw": trigw,
                "maskd": maskd,
            }
        )
    return in_maps


def kernel(x, w_attn, w_proj):
    global LAST_RUN
    nc = _get_nc()
    in_maps = _prep_inputs(x, w_attn, w_proj)
    res = run_bass_kernel_spmd(nc, in_maps, core_ids=list(range(N_CORES)))
    LAST_RUN = res
    out = np.empty((B, T, C), dtype=np.float32)
    for b in range(B):
        acc = res.results[TP * b]["outT"] + res.results[TP * b + 1]["outT"]
        out[b] = acc.T
    return out



# revision 4
# speedup vs baseline: 1.1404x; 1.1404x over previous
"""Causal self-attention (GQA + RoPE) Trainium2 kernel, 8-way sharded.

Sharding: DP=4 over batch x TP=2 over kv-head groups (2 kv heads + their
8 q heads per group).  Each core computes its batch's qkv projection for
its head group, causal attention, and a partial c_proj (columns of
w_proj for its head group).  Host sums the two partial c_proj outputs
per batch.

Everything on-chip runs transposed ([feature, token] layout) so matmuls
contract along partitions; host transposes inputs/outputs.

Projection matmuls (qkv, v, c_proj) run as fp8e4 DoubleRow "triple-MMs":
each operand A is sent as A_hi + A_lo (both e4m3; hi = rounded value,
lo = rounded residual), and each pair of 128-deep contraction chunks is
computed with three DoubleRow matmuls
    hi*hi + hi*lo + lo*hi          (lo*lo ~ 0.06% -- dropped)
at half-rate each, i.e. 0.75x the bf16 cost with ~bf16 accuracy.
Weights are prescaled by 64 so their sigma ~ 1/45 lands in e4m3 normal
range; the inverse scale is folded into the RoPE trig tables / the
PSUM->SBUF copies.  Attention itself (QK, AV, softmax) stays bf16.

Pipeline: the attention inner loop leaves PE slack while ACT churns
exps, so the q/k projection work for head h+1 is interleaved into the
PE stream of head h's attention; head 7 interleaves c_proj instead.
The AV/denominator matmuls for tile kt are emitted two k-tiles late
(lag-2 software pipeline) so the exp -> mask chain never stalls the PE.
Causal masking only touches the one 128x128 triangle block per diagonal
tile; the valid column range of a diagonal tile is computed mask-free.

RoPE: w_attn q/k rows are permuted per-head to [even dims; odd dims] so
rotation pairs land at partition f and f+64 of the qkv psum tile:
  P  = ps * [c; c] (SBUF),  P2 = ps * [s; s] (PSUM)
  out[0:64]   = P[0:64]  - P2[64:128]
  out[64:128] = P2[0:64] + P[64:128]
(each combine reads one SBUF + one PSUM operand, which may sit at
different base partitions; two SBUF operands may not).

Softmax: att^T tiles ([k, q] layout) are exp'd on ACT without
max-subtraction (logits are O(6), fp32-safe).  Denominators: groups of
4 e-tiles are tree-summed on DVE and hit with one ones-column matmul
per group (deferred into the next group's PE stream); the per-q
reciprocal is broadcast down partitions with a f32r outer-product
matmul, also deferred one q-tile.
"""

import math

import numpy as np
import ml_dtypes

import concourse.bass as bass
import concourse.mybir as mybir
import concourse.tile as tile
from concourse import bacc
from concourse.bass_utils import run_bass_kernel_spmd

ALU = mybir.AluOpType
AF = mybir.ActivationFunctionType
F32 = mybir.dt.float32
F32R = mybir.dt.float32r
BF16 = mybir.dt.bfloat16
FP8 = mybir.dt.float8e4
DR = mybir.MatmulPerfMode.DoubleRow
BF = ml_dtypes.bfloat16
E4 = ml_dtypes.float8_e4m3

# problem shape (hardcoded per contest rules)
B, T, C = 4, 2048, 2048
N_HEAD, N_KV_HEAD, HD = 16, 4, 128
ROPE_THETA = 10000.0

TP = 2            # head-group shards
DP = 4            # batch shards
HQ = N_HEAD // TP         # 8 q heads per core
HKV = N_KV_HEAD // TP     # 2 kv heads per core
NREP = N_HEAD // N_KV_HEAD  # 4
QK_ROWS = (HQ + HKV) * HD   # 1280
KC = C // 128     # 16 contraction tiles
NQ = T // 512     # 4 token strips
MQK = QK_ROWS // 128  # 10 feature tiles (8 q heads + 2 kv heads)
FM = C // 128     # 16 output feature tiles
SCALE = 1.0 / math.sqrt(HD)
WS = 64.0         # weight prescale for e4m3 range

N_CORES = 8

_NC = None        # cached compiled Bass module
LAST_RUN = None   # BassKernelResults of the most recent kernel() call


def build_nc():
    nc = bacc.Bacc(None, target_bir_lowering=False, debug=False)

    xhi = nc.declare_dram_parameter("xhi", [128, KC, T], FP8, isOutput=False)
    xlo = nc.declare_dram_parameter("xlo", [128, KC, T], FP8, isOutput=False)
    wqk_hl = nc.declare_dram_parameter("wqk_hl", [MQK, 128, 2 * KC * 128], FP8, isOutput=False)
    wv_hi = nc.declare_dram_parameter("wv_hi", [128, KC * HKV * HD], FP8, isOutput=False)
    wv_lo = nc.declare_dram_parameter("wv_lo", [128, KC * HKV * HD], FP8, isOutput=False)
    wp_hl = nc.declare_dram_parameter("wp_hl", [FM, 128, 2 * HQ * 128], FP8, isOutput=False)
    trigf = nc.declare_dram_parameter("trigf", [128, T], F32, isOutput=False)  # [c;c]/WS
    trigw = nc.declare_dram_parameter("trigw", [128, T], F32, isOutput=False)  # [s;s]/WS
    maskd = nc.declare_dram_parameter("maskd", [128, 128], BF16, isOutput=False)
    outT = nc.declare_dram_parameter("outT", [C, T], BF16, isOutput=True)

    with tile.TileContext(nc) as tc:
        with (
            tc.tile_pool(name="const", bufs=1) as const,
            tc.tile_pool(name="persist", bufs=1) as persist,
            tc.tile_pool(name="eb", bufs=8) as eb,
            tc.tile_pool(name="gag", bufs=2) as gag,
            tc.tile_pool(name="rb", bufs=1) as rb,
            tc.tile_pool(name="ytp", bufs=2) as ytp,
            tc.tile_pool(name="psS", bufs=2, space="PSUM") as psS,
            tc.tile_pool(name="psY", bufs=2, space="PSUM") as psY,
            tc.tile_pool(name="psD", bufs=2, space="PSUM") as psD,
        ):
            trigf_sb = const.tile([128, T], F32, name="trigf")
            trigw_sb = const.tile([128, T], F32, name="trigw")
            mask_sb = const.tile([128, 128], BF16, name="mask")
            ones_col = const.tile([128, 1], BF16, name="onec")
            ones_row_f = const.tile([1, 128], F32, name="onerf")
            ones_row = const.tile([1, 128], F32R, name="oner")

            qrot = [persist.tile([128, T], BF16, name=f"qrot{h}") for h in range(HQ)]
            krot = [persist.tile([128, T], BF16, name=f"krot{h}") for h in range(HKV)]
            v_sb = persist.tile([128, T // 128, HKV * HD], BF16, name="vtok")
            yt_hi = persist.tile([128, HQ, T], FP8, name="ythi")
            yt_lo = persist.tile([128, HQ, T], FP8, name="ytlo")

            state = {"pending": None, "pending_ones": None}

            def finalize(h, qj, ps_y, ps_d):
                qsl = bass.ts(qj, 512)
                rec_f = rb.tile([1, 512], F32, name="recf")
                rec_r = rb.tile([1, 512], F32R, name="recr")
                r_sb = rb.tile([128, 512], F32, name="r")
                nc.vector.reciprocal(rec_f[:], ps_d[:])
                with nc.allow_low_precision("f32r recip broadcast"):
                    nc.vector.tensor_copy(rec_r[:], rec_f[:])
                ps_r = psS.tile([128, 512], F32, name="pss")
                nc.tensor.matmul(
                    ps_r[:], ones_row[:], rec_r[:], start=True, stop=True
                )
                nc.scalar.copy(r_sb[:], ps_r[:])
                tmp = ytp.tile([128, 512], BF16, name="ytmp")
                nc.vector.tensor_tensor(tmp[:], ps_y[:], r_sb[:], ALU.mult)
                nc.gpsimd.tensor_copy(yt_hi[:, h, qsl], tmp[:])
                nc.gpsimd.tensor_tensor(
                    yt_lo[:, h, qsl], tmp[:], yt_hi[:, h, qsl], ALU.subtract
                )

            def flush_ones():
                if state["pending_ones"] is not None:
                    po, st, sp, pd = state["pending_ones"]
                    nc.tensor.matmul(pd[:], ones_col[:], po[:], start=st, stop=sp)
                    state["pending_ones"] = None

            def emit_qj(h, qj, pop):
                """Attention for (h, qj): QK tiles, exp, mask, AV, denominators.

                AV/denominator matmuls run two k-tiles behind QK/exp (lag-2
                software pipeline).  `pop()` is called once per k-tile to
                interleave filler PE work.  Returns the (ps_y, ps_d)
                accumulators (not yet finalized).
                """
                kvh = h // NREP
                vs = lambda kt: v_sb[:, kt, kvh * HD : (kvh + 1) * HD]
                ps_y = psY.tile([128, 512], F32, name="psy")
                ps_d = psD.tile([1, 512], F32, name="psd")
                nkt = 4 * qj + 4
                g0 = ga = g2 = None
                hist = []

                def emit_av(e, d, lo, kt):
                    first = kt == 0
                    last = kt == nkt - 1
                    mid = lo + 128
                    if d >= 0 and mid < 512:
                        # valid columns first (no mask dependency), then the
                        # masked 128-wide triangle block
                        nc.tensor.matmul(
                            ps_y[:, mid:512], vs(kt), e[:, mid:512],
                            start=first, stop=False,
                        )
                        nc.tensor.matmul(
                            ps_y[:, lo:mid], vs(kt), e[:, lo:mid],
                            start=False, stop=last,
                        )
                    else:
                        nc.tensor.matmul(
                            ps_y[:, lo:512], vs(kt), e[:, lo:512],
                            start=first, stop=last,
                        )
                    if d >= 0:
                        if d == 0:
                            flush_ones()
                        nc.tensor.matmul(
                            ps_d[:, lo:512], ones_col[:], e[:, lo:512],
                            start=(qj == 0 and kt == 0), stop=last,
                        )

                for kt in range(nkt):
                    d = kt - 4 * qj
                    # diagonal tile d has valid q-columns only in [128d, 512)
                    lo = 128 * d if d > 0 else 0
                    qlo = qj * 512 + lo
                    ps_s = psS.tile([128, 512], F32, name="pss")
                    nc.tensor.matmul(
                        ps_s[:, lo:512],
                        krot[kvh][:, kt * 128 : (kt + 1) * 128],
                        qrot[h][:, qlo : (qj + 1) * 512],
                        start=True,
                        stop=True,
                    )
                    e = eb.tile([128, 512], BF16, name="e")
                    nc.scalar.activation(
                        e[:, lo:512], ps_s[:, lo:512], AF.Exp, scale=SCALE
                    )
                    if d >= 0:
                        # mask only the 128x128 triangle block
                        nc.vector.tensor_tensor(
                            e[:, lo : lo + 128], e[:, lo : lo + 128],
                            mask_sb[:], ALU.mult,
                        )
                    else:
                        # full groups: tree-sum 4 e-tiles on DVE, one deferred
                        # ones-matmul per group (emitted in a later PE slot so
                        # the PE never waits on the DVE adds).
                        ph = kt % 4
                        if ph == 0:
                            g0 = e
                        elif ph == 1:
                            ga = gag.tile([128, 512], BF16, name="ga")
                            nc.vector.tensor_tensor(ga[:], g0[:], e[:], ALU.add)
                        elif ph == 2:
                            g2 = e
                        else:
                            gs = gag.tile([128, 512], BF16, name="gs")
                            nc.vector.tensor_tensor(gs[:], g2[:], e[:], ALU.add)
                            nc.vector.tensor_tensor(gs[:], gs[:], ga[:], ALU.add)
                            flush_ones()
                            grp = kt // 4
                            state["pending_ones"] = (gs, grp == 0, False, ps_d)
                    hist.append((e, d, lo, kt))
                    if len(hist) > 2:
                        emit_av(*hist.pop(0))
                    pop(kt)
                for item in hist:
                    emit_av(*item)
                return ps_y, ps_d

            # ======== projection machinery (lives through heads 0..6) ========
            with (
                tc.tile_pool(name="xa", bufs=1) as xa,
                tc.tile_pool(name="wm", bufs=3) as wm,
                tc.tile_pool(name="ta", bufs=1) as ta,
                tc.tile_pool(name="psA", bufs=1, space="PSUM") as psA,
                tc.tile_pool(name="psP2", bufs=1, space="PSUM") as psP2,
            ):
                xs_hi = xa.tile([128, KC, T], FP8, name="xshi")
                xs_lo = xa.tile([128, KC, T], FP8, name="xslo")

                def load_wm(m):
                    w = wm.tile([128, 2, KC, 128], FP8, name="wmhl")
                    nc.sync.dma_start(
                        w[:], wqk_hl[m].rearrange("p (l kc c) -> p l kc c", l=2, kc=KC)
                    )
                    return w

                def proj_mms(ps, w, nsl):
                    """Triple-MM qkv projection chunk stream for one strip."""
                    for p in range(KC // 2):
                        sl = slice(2 * p, 2 * p + 2)
                        nc.tensor.matmul(
                            ps[:], w[:, 0, sl, :], xs_hi[:, sl, nsl],
                            start=(p == 0), stop=False, perf_mode=DR,
                        )
                        nc.tensor.matmul(
                            ps[:], w[:, 0, sl, :], xs_lo[:, sl, nsl],
                            start=False, stop=False, perf_mode=DR,
                        )
                        nc.tensor.matmul(
                            ps[:], w[:, 1, sl, :], xs_hi[:, sl, nsl],
                            start=False, stop=(p == KC // 2 - 1), perf_mode=DR,
                        )
                        yield

                def rope_ops(m, n, ps):
                    """The four RoPE ops for one (feature tile, strip) pair."""
                    dst = qrot[m] if m < HQ else krot[m - HQ]
                    nsl = bass.ts(n, 512)
                    pt = ta.tile([128, 512], F32, name="pt")
                    p2 = psP2.tile([128, 512], F32, name="p2")
                    yield nc.vector.tensor_tensor(
                        pt[:], ps[:], trigf_sb[:, nsl], ALU.mult
                    )
                    yield nc.vector.tensor_tensor(
                        p2[:], ps[:], trigw_sb[:, nsl], ALU.mult
                    )
                    yield nc.vector.tensor_tensor(
                        dst[0:64, nsl], pt[0:64, :], p2[64:128, :], ALU.subtract
                    )
                    yield nc.vector.tensor_tensor(
                        dst[64:128, nsl], p2[0:64, :], pt[64:128, :], ALU.add
                    )

                def a_stream(m, pool):
                    w = load_wm(m)
                    yield
                    for n in range(NQ):
                        nsl = bass.ts(n, 512)
                        ps = pool.tile([128, 512], F32, name="psA")
                        yield from proj_mms(ps[:], w, nsl)
                        for _ in rope_ops(m, n, ps):
                            yield

                # ---- A0: v projection + k heads + q head 0 (pure PE phase) ----
                with tc.tile_pool(name="wvp", bufs=1) as wvp:
                    wv_sbh = wvp.tile([128, KC, HKV * HD], FP8, name="wvh")
                    wv_sbl = wvp.tile([128, KC, HKV * HD], FP8, name="wvl")
                    nc.sync.dma_start(
                        wv_sbh[:], wv_hi.rearrange("p (kc c) -> p kc c", kc=KC)
                    )
                    nc.sync.dma_start(
                        wv_sbl[:], wv_lo.rearrange("p (kc c) -> p kc c", kc=KC)
                    )
                    for i in range(4):
                        ksl = slice(4 * i, 4 * i + 4)
                        nc.sync.dma_start(xs_hi[:, ksl, bass.ts(0, 512)],
                                          xhi[:, ksl, bass.ts(0, 512)])
                        nc.sync.dma_start(xs_lo[:, ksl, bass.ts(0, 512)],
                                          xlo[:, ksl, bass.ts(0, 512)])
                    nc.sync.dma_start(trigf_sb[:], trigf[:])
                    nc.sync.dma_start(trigw_sb[:], trigw[:])
                    nc.sync.dma_start(mask_sb[:], maskd[:])
                    nc.vector.memset(ones_col[:], 1.0)
                    nc.vector.memset(ones_row_f[:], 1.0)
                    with nc.allow_low_precision("f32r ones for recip broadcast"):
                        nc.vector.tensor_copy(ones_row[:], ones_row_f[:])
                    wk0 = load_wm(HQ)
                    wk1 = load_wm(HQ + 1)
                    wq0 = load_wm(0)
                    for n in range(NQ):
                        nsl = bass.ts(n, 512)
                        if n + 1 < NQ:
                            nsl_next = bass.ts(n + 1, 512)
                            for i in range(4):
                                ksl = slice(4 * i, 4 * i + 4)
                                nc.sync.dma_start(xs_hi[:, ksl, nsl_next],
                                                  xhi[:, ksl, nsl_next])
                                nc.sync.dma_start(xs_lo[:, ksl, nsl_next],
                                                  xlo[:, ksl, nsl_next])
                        for tt in range(4 * n, 4 * n + 4):
                            # reuse the attention-phase psum slots during A0
                            tsl = slice(tt * 128, (tt + 1) * 128)
                            psv = psS.tile([128, 512], F32, name="pss")[
                                :, : HKV * HD
                            ]
                            for p in range(KC // 2):
                                sl = slice(2 * p, 2 * p + 2)
                                nc.tensor.matmul(
                                    psv[:], xs_hi[:, sl, tsl], wv_sbh[:, sl, :],
                                    start=(p == 0), stop=False, perf_mode=DR,
                                )
                                nc.tensor.matmul(
                                    psv[:], xs_hi[:, sl, tsl], wv_sbl[:, sl, :],
                                    start=False, stop=False, perf_mode=DR,
                                )
                                nc.tensor.matmul(
                                    psv[:], xs_lo[:, sl, tsl], wv_sbh[:, sl, :],
                                    start=False, stop=(p == KC // 2 - 1),
                                    perf_mode=DR,
                                )
                            nc.scalar.activation(
                                v_sb[:, tt, :], psv[:], AF.Copy, scale=1.0 / WS
                            )
                        for m, w in ((HQ, wk0), (HQ + 1, wk1), (0, wq0)):
                            ps = psY.tile([128, 512], F32, name="psy")
                            for _ in proj_mms(ps[:], w, nsl):
                                pass
                            for _ in rope_ops(m, n, ps):
                                pass

                # ---- heads 0..6: attention + next head's projection ----
                for h in range(HQ - 1):
                    agen = a_stream(h + 1, psA)

                    def pop(kt, agen=agen):
                        next(agen, None)
                        if kt < 5 or kt >= 10:
                            next(agen, None)

                    for qj in range(NQ):
                        ps_y, ps_d = emit_qj(h, qj, pop)
                        if state["pending"] is not None:
                            finalize(*state["pending"])
                        state["pending"] = (h, qj, ps_y, ps_d)
                    for _ in agen:
                        pass

            # ---- head 7: attention + output projection interleaved ----
            with (
                tc.tile_pool(name="wpc", bufs=4) as wpc,
                tc.tile_pool(name="obp", bufs=3) as obp,
                tc.tile_pool(name="psO", bufs=2, space="PSUM") as psO,
            ):
                def load_wp(fm):
                    wc = wpc.tile([128, 2, HQ, 128], FP8, name="wc")
                    nc.sync.dma_start(
                        wc[:], wp_hl[fm].rearrange("p (l h c) -> p l h c", l=2, h=HQ)
                    )
                    return wc

                def c_stream(n):
                    """Output projection for token strip n (16 feature tiles)."""
                    nsl = bass.ts(n, 512)
                    wcs = [load_wp(0), load_wp(1)]
                    for fm in range(FM):
                        wc = wcs.pop(0)
                        yield
                        ps_o = psO.tile([128, 512], F32, name="pso")
                        for p in range(HQ // 2):
                            sl = slice(2 * p, 2 * p + 2)
                            nc.tensor.matmul(
                                ps_o[:], wc[:, 0, sl, :], yt_hi[:, sl, nsl],
                                start=(p == 0), stop=False, perf_mode=DR,
                            )
                            nc.tensor.matmul(
                                ps_o[:], wc[:, 0, sl, :], yt_lo[:, sl, nsl],
                                start=False, stop=False, perf_mode=DR,
                            )
                            nc.tensor.matmul(
                                ps_o[:], wc[:, 1, sl, :], yt_hi[:, sl, nsl],
                                start=False, stop=(p == HQ // 2 - 1),
                                perf_mode=DR,
                            )
                            if p == 0 and fm + 2 < FM:
                                wcs.append(load_wp(fm + 2))
                            yield
                        ob = obp.tile([128, 512], BF16, name="ob")
                        nc.scalar.activation(ob[:], ps_o[:], AF.Copy, scale=1.0 / WS)
                        nc.sync.dma_start(
                            outT[fm * 128 : (fm + 1) * 128, nsl], ob[:]
                        )
                        yield

                cgens = []

                _end = object()

                def pop7(kt):
                    for _ in range(2):
                        while cgens:
                            if next(cgens[0], _end) is _end:
                                cgens.pop(0)
                                continue
                            break

                for qj in range(NQ):
                    ps_y, ps_d = emit_qj(HQ - 1, qj, pop7)
                    flush_ones()
                    if state["pending"] is not None:
                        finalize(*state["pending"])
                        state["pending"] = None
                    finalize(HQ - 1, qj, ps_y, ps_d)
                    cgens.append(c_stream(qj))
                # drain remaining output projection
                for g in cgens:
                    for _ in g:
                        pass

    nc.compile()
    return nc


def _get_nc():
    global _NC
    if _NC is None:
        _NC = build_nc()
    return _NC


def _split_hilo(a):
    """a = hi + lo with both parts e4m3."""
    hi = a.astype(E4)
    lo = (a - hi.astype(np.float32)).astype(E4)
    return hi, lo


def _prep_inputs(x, w_attn, w_proj):
    """Build the 8 per-core input maps from the full-problem arrays."""
    perm = np.concatenate([np.arange(0, HD, 2), np.arange(1, HD, 2)])

    f = np.arange(64, dtype=np.float64)
    inv = ROPE_THETA ** (-2.0 * f / HD)
    ang = inv[:, None] * np.arange(T, dtype=np.float64)[None, :]
    trigc = (np.cos(ang) / WS).astype(np.float32)
    trigs = (np.sin(ang) / WS).astype(np.float32)
    trigf = np.ascontiguousarray(np.concatenate([trigc, trigc], axis=0))
    trigw = np.ascontiguousarray(np.concatenate([trigs, trigs], axis=0))

    kk = np.arange(128)[:, None]
    qq = np.arange(128)[None, :]
    maskd = (kk <= qq).astype(BF)  # [128 k, 128 q] lower-triangle-valid

    w_attn = np.asarray(w_attn)
    w_proj = np.asarray(w_proj)
    x = np.asarray(x)

    in_maps = []
    for core in range(N_CORES):
        b, g = core // TP, core % TP
        # x features chunked: xhi[p, kc, t] = x[b].T[kc*128+p, t]
        xT = np.ascontiguousarray(x[b].T)  # [C, T] f32
        x_hi, x_lo = _split_hilo(xT)
        xhi = np.ascontiguousarray(x_hi.reshape(KC, 128, T).transpose(1, 0, 2))
        xlo = np.ascontiguousarray(x_lo.reshape(KC, 128, T).transpose(1, 0, 2))

        qrows = []
        for h in range(HQ):
            gh = g * HQ + h
            qrows.append(gh * HD + perm)
        for kv in range(HKV):
            gk = g * HKV + kv
            qrows.append(N_HEAD * HD + gk * HD + perm)
        qrows = np.concatenate(qrows)
        wqk = w_attn[qrows] * WS  # [1280, C] f32
        # stationary layout: wqk3[m, p, kc*128+col] = wqk[m*128+col, kc*128+p]
        wqk3 = np.ascontiguousarray(
            wqk.reshape(MQK, 128, KC, 128).transpose(0, 3, 2, 1).reshape(MQK, 128, C)
        )
        wqk3_hi, wqk3_lo = _split_hilo(wqk3)
        wqk_hl = np.ascontiguousarray(
            np.stack([wqk3_hi, wqk3_lo], axis=2).reshape(MQK, 128, 2 * C)
        )

        vrows = np.concatenate(
            [
                (N_HEAD + N_KV_HEAD) * HD + (g * HKV + kv) * HD + np.arange(HD)
                for kv in range(HKV)
            ]
        )
        wv = w_attn[vrows] * WS  # [256, C]
        # wv3[p, kc*256+c] = wv[c, kc*128+p]
        wv3 = np.ascontiguousarray(
            wv.reshape(HKV * HD, KC, 128).transpose(2, 1, 0).reshape(128, KC * HKV * HD)
        )
        wv3_hi, wv3_lo = _split_hilo(wv3)

        cols = np.arange(g * HQ * HD, (g + 1) * HQ * HD)
        wpg = w_proj[:, cols] * WS  # [C, 1024], rows = out features
        # wp5[fm, d, h*128+p] = wpg[fm*128+p, h*128+d]
        wp5 = np.ascontiguousarray(
            wpg.T.reshape(HQ, 128, FM, 128).transpose(2, 1, 0, 3).reshape(FM, 128, HQ * 128)
        )
        wp5_hi, wp5_lo = _split_hilo(wp5)
        wp_hl = np.ascontiguousarray(
            np.stack([wp5_hi, wp5_lo], axis=2).reshape(FM, 128, 2 * HQ * 128)
        )

        in_maps.append(
            {
                "xhi": xhi,
                "xlo": xlo,
                "wqk_hl": wqk_hl,
                "wv_hi": wv3_hi,
                "wv_lo": wv3_lo,
                "wp_hl": wp_hl,
                "trigf": trigf,
                "trigw": trigw,
                "maskd": maskd,
            }
        )
    return in_maps


def kernel(x, w_attn, w_proj):
    global LAST_RUN
    nc = _get_nc()
    in_maps = _prep_inputs(x, w_attn, w_proj)
    res = run_bass_kernel_spmd(nc, in_maps, core_ids=list(range(N_CORES)))
    LAST_RUN = res
    out = np.empty((B, T, C), dtype=np.float32)
    for b in range(B):
        acc = (
            res.results[TP * b]["outT"].astype(np.float32)
            + res.results[TP * b + 1]["outT"].astype(np.float32)
        )
        out[b] = acc.T
    return out


# revision 22
# speedup vs baseline: 1.2040x; 1.0558x over previous
"""Causal self-attention (GQA + RoPE) Trainium2 kernel, 8-way sharded.

Sharding: DP=4 over batch x TP=2 over kv-head groups (2 kv heads + their
8 q heads per group).  Each core computes its batch's qkv projection for
its head group, causal attention, and a partial c_proj (columns of
w_proj for its head group).  Host sums the two partial c_proj outputs
per batch.

Everything on-chip runs transposed ([feature, token] layout) so matmuls
contract along partitions; host transposes inputs/outputs.

Projection matmuls (qkv, v, c_proj) run as fp8e4 DoubleRow "triple-MMs":
each operand A is sent as A_hi + A_lo (both e4m3; hi = rounded value,
lo = rounded residual), and each pair of 128-deep contraction chunks is
computed with three DoubleRow matmuls
    hi*hi + hi*lo + lo*hi          (lo*lo ~ 0.06% -- dropped)
at half-rate each, i.e. 0.75x the bf16 cost with ~bf16 accuracy.
Weights are prescaled by 64 so their sigma ~ 1/45 lands in e4m3 normal
range; the inverse scale is folded into the RoPE trig tables / the
PSUM->SBUF copies.  Attention itself (QK, AV, softmax) stays bf16.

Pipeline: the attention inner loop leaves PE slack while ACT churns
exps, so the q/k projection work for head h+1 is interleaved into the
PE stream of head h's attention; head 7 interleaves c_proj instead.
The AV/denominator matmuls for tile kt are emitted two k-tiles late
(lag-2 software pipeline) so the exp -> mask chain never stalls the PE.
Causal masking only touches the one 128x128 triangle block per diagonal
tile; the valid column range of a diagonal tile is computed mask-free.

RoPE: w_attn q/k rows are permuted per-head to [even dims; odd dims] so
rotation pairs land at partition f and f+64 of the qkv psum tile:
  P  = ps * [c; c] (SBUF),  P2 = ps * [s; s] (PSUM)
  out[0:64]   = P[0:64]  - P2[64:128]
  out[64:128] = P2[0:64] + P[64:128]
(each combine reads one SBUF + one PSUM operand, which may sit at
different base partitions; two SBUF operands may not).

Softmax: att^T tiles ([k, q] layout) are exp'd on ACT without
max-subtraction (logits are O(6), fp32-safe).  Denominators: groups of
4 e-tiles are tree-summed on DVE and hit with one ones-column matmul
per group (deferred into the next group's PE stream); the per-q
reciprocal is broadcast down partitions with a f32r outer-product
matmul, also deferred one q-tile.
"""

import math

import numpy as np
import ml_dtypes

import concourse.bass as bass
import concourse.mybir as mybir
import concourse.tile as tile
from concourse import bacc
from concourse.bass_utils import run_bass_kernel_spmd

ALU = mybir.AluOpType
AF = mybir.ActivationFunctionType
F32 = mybir.dt.float32
F32R = mybir.dt.float32r
BF16 = mybir.dt.bfloat16
FP8 = mybir.dt.float8e4
DR = mybir.MatmulPerfMode.DoubleRow
BF = ml_dtypes.bfloat16
E4 = ml_dtypes.float8_e4m3

# problem shape (hardcoded per contest rules)
B, T, C = 4, 2048, 2048
N_HEAD, N_KV_HEAD, HD = 16, 4, 128
ROPE_THETA = 10000.0

TP = 2            # head-group shards
DP = 4            # batch shards
HQ = N_HEAD // TP         # 8 q heads per core
HKV = N_KV_HEAD // TP     # 2 kv heads per core
NREP = N_HEAD // N_KV_HEAD  # 4
QK_ROWS = (HQ + HKV) * HD   # 1280
KC = C // 128     # 16 contraction tiles
NQ = T // 512     # 4 token strips
MQK = QK_ROWS // 128  # 10 feature tiles (8 q heads + 2 kv heads)
FM = C // 128     # 16 output feature tiles
SCALE = 1.0 / math.sqrt(HD)
WS = 64.0         # weight prescale for e4m3 range

N_CORES = 8

_NC = None        # cached compiled Bass module
LAST_RUN = None   # BassKernelResults of the most recent kernel() call


def build_nc():
    nc = bacc.Bacc(None, target_bir_lowering=False, debug=False)

    xhi = nc.declare_dram_parameter("xhi", [128, KC, T], FP8, isOutput=False)
    xlo = nc.declare_dram_parameter("xlo", [128, KC, T], FP8, isOutput=False)
    wqk_hl = nc.declare_dram_parameter("wqk_hl", [MQK, 128, 2 * KC * 128], FP8, isOutput=False)
    wv_hi = nc.declare_dram_parameter("wv_hi", [128, KC * HKV * HD], FP8, isOutput=False)
    wv_lo = nc.declare_dram_parameter("wv_lo", [128, KC * HKV * HD], FP8, isOutput=False)
    wp_hl = nc.declare_dram_parameter("wp_hl", [FM, 128, 2 * HQ * 128], FP8, isOutput=False)
    trigf = nc.declare_dram_parameter("trigf", [128, T], F32, isOutput=False)  # [c;c]/WS
    trigw = nc.declare_dram_parameter("trigw", [128, T], F32, isOutput=False)  # [s;s]/WS
    maskd = nc.declare_dram_parameter("maskd", [128, 128], BF16, isOutput=False)
    outT = nc.declare_dram_parameter("outT", [C, T], BF16, isOutput=True)

    with tile.TileContext(nc) as tc:
        with (
            tc.tile_pool(name="const", bufs=1) as const,
            tc.tile_pool(name="persist", bufs=1) as persist,
            tc.tile_pool(name="eb", bufs=8) as eb,
            tc.tile_pool(name="gag", bufs=2) as gag,
            tc.tile_pool(name="rb", bufs=1) as rb,
            tc.tile_pool(name="ytp", bufs=2) as ytp,
            tc.tile_pool(name="psS", bufs=2, space="PSUM") as psS,
            tc.tile_pool(name="psY", bufs=2, space="PSUM") as psY,
            tc.tile_pool(name="psD", bufs=2, space="PSUM") as psD,
        ):
            trigf_sb = const.tile([128, T], F32, name="trigf")
            trigw_sb = const.tile([128, T], F32, name="trigw")
            mask_sb = const.tile([128, 128], BF16, name="mask")
            ones_mat = const.tile([128, 128], BF16, name="onem")

            qrot = [persist.tile([128, T], BF16, name=f"qrot{h}") for h in range(HQ)]
            krot = [persist.tile([128, T], BF16, name=f"krot{h}") for h in range(HKV)]
            v_sb = persist.tile([128, T // 128, HKV * HD], BF16, name="vtok")
            yt_hi = persist.tile([128, HQ, T], FP8, name="ythi")
            yt_lo = persist.tile([128, HQ, T], FP8, name="ytlo")

            state = {"pending": None, "pending_ones": None}

            def finalize(h, qj, ps_y, ps_d):
                qsl = bass.ts(qj, 512)
                r_sb = rb.tile([128, 512], F32, name="r")
                nc.vector.reciprocal(r_sb[:], ps_d[:])
                tmp = ytp.tile([128, 512], BF16, name="ytmp")
                nc.vector.tensor_tensor(tmp[:], ps_y[:], r_sb[:], ALU.mult)
                nc.gpsimd.tensor_copy(yt_hi[:, h, qsl], tmp[:])
                nc.gpsimd.tensor_tensor(
                    yt_lo[:, h, qsl], tmp[:], yt_hi[:, h, qsl], ALU.subtract
                )

            def flush_ones():
                if state["pending_ones"] is not None:
                    po, st, sp, pd = state["pending_ones"]
                    nc.tensor.matmul(pd[:], ones_mat[:], po[:], start=st, stop=sp)
                    state["pending_ones"] = None

            def strip_tiles(h, qj, res):
                """Generator: attention for (h, qj), yielding once per k-tile.

                AV/denominator matmuls run two k-tiles behind QK/exp (lag-2
                software pipeline).  The (ps_y, ps_d) accumulators are stored
                in res[qj] (finalized by the caller).
                """
                kvh = h // NREP
                vs = lambda kt: v_sb[:, kt, kvh * HD : (kvh + 1) * HD]
                ps_y = psY.tile([128, 512], F32, name="psy")
                ps_d = psD.tile([128, 512], F32, name="psd")
                res[qj] = (ps_y, ps_d)
                nkt = 4 * qj + 4
                g0 = ga = g2 = None
                hist = []

                def emit_av(e, d, lo, kt):
                    first = kt == 0
                    last = kt == nkt - 1
                    mid = lo + 128
                    if d >= 0 and mid < 512:
                        # valid columns first (no mask dependency), then the
                        # masked 128-wide triangle block
                        nc.tensor.matmul(
                            ps_y[:, mid:512], vs(kt), e[:, mid:512],
                            start=first, stop=False,
                        )
                        nc.tensor.matmul(
                            ps_y[:, lo:mid], vs(kt), e[:, lo:mid],
                            start=False, stop=last,
                        )
                    else:
                        nc.tensor.matmul(
                            ps_y[:, lo:512], vs(kt), e[:, lo:512],
                            start=first, stop=last,
                        )
                    if d >= 0:
                        if d == 0:
                            flush_ones()
                        nc.tensor.matmul(
                            ps_d[:, lo:512], ones_mat[:], e[:, lo:512],
                            start=(qj == 0 and kt == 0), stop=last,
                        )

                for kt in range(nkt):
                    d = kt - 4 * qj
                    # diagonal tile d has valid q-columns only in [128d, 512)
                    lo = 128 * d if d > 0 else 0
                    qlo = qj * 512 + lo
                    ps_s = psS.tile([128, 512], F32, name="pss")
                    nc.tensor.matmul(
                        ps_s[:, lo:512],
                        krot[kvh][:, kt * 128 : (kt + 1) * 128],
                        qrot[h][:, qlo : (qj + 1) * 512],
                        start=True,
                        stop=True,
                    )
                    e = eb.tile([128, 512], BF16, name="e")
                    nc.scalar.activation(
                        e[:, lo:512], ps_s[:, lo:512], AF.Exp, scale=SCALE
                    )
                    if d >= 0:
                        # mask only the 128x128 triangle block (on GPSIMD --
                        # the lag-2 AV emission gives the chain plenty of slack)
                        nc.gpsimd.tensor_tensor(
                            e[:, lo : lo + 128], e[:, lo : lo + 128],
                            mask_sb[:], ALU.mult,
                        )
                    else:
                        # full groups: tree-sum 4 e-tiles (first add on GPSIMD,
                        # rest on DVE), one deferred ones-matmul per group
                        # (emitted in a later PE slot so the PE never waits on
                        # the adds).
                        ph = kt % 4
                        if ph == 0:
                            g0 = e
                        elif ph == 1:
                            ga = gag.tile([128, 512], BF16, name="ga")
                            nc.gpsimd.tensor_tensor(ga[:], g0[:], e[:], ALU.add)
                        elif ph == 2:
                            g2 = e
                        else:
                            gs = gag.tile([128, 512], BF16, name="gs")
                            nc.vector.tensor_tensor(gs[:], g2[:], e[:], ALU.add)
                            nc.vector.tensor_tensor(gs[:], gs[:], ga[:], ALU.add)
                            flush_ones()
                            grp = kt // 4
                            state["pending_ones"] = (gs, grp == 0, False, ps_d)
                    hist.append((e, d, lo, kt))
                    if len(hist) > 2:
                        emit_av(*hist.pop(0))
                    yield
                for item in hist:
                    emit_av(*item)

            def emit_qj(h, qj, pop):
                """Attention for (h, qj), with pop() called once per k-tile."""
                res = {}
                gen = strip_tiles(h, qj, res)
                kt = 0
                while next(gen, _END) is not _END:
                    pop(kt)
                    kt += 1
                return res[qj]

            _END = object()

            # ======== projection machinery (lives through heads 0..6) ========
            with (
                tc.tile_pool(name="xa", bufs=1) as xa,
                tc.tile_pool(name="wm", bufs=3) as wm,
                tc.tile_pool(name="ta", bufs=1) as ta,
                tc.tile_pool(name="psA", bufs=1, space="PSUM") as psA,
                tc.tile_pool(name="psP2", bufs=1, space="PSUM") as psP2,
            ):
                xs_hi = xa.tile([128, KC, T], FP8, name="xshi")
                xs_lo = xa.tile([128, KC, T], FP8, name="xslo")

                def load_wm(m):
                    w = wm.tile([128, 2, KC, 128], FP8, name="wmhl")
                    nc.sync.dma_start(
                        w[:], wqk_hl[m].rearrange("p (l kc c) -> p l kc c", l=2, kc=KC)
                    )
                    return w

                def proj_mms(ps, w, nsl):
                    """Triple-MM qkv projection chunk stream for one strip."""
                    for p in range(KC // 2):
                        sl = slice(2 * p, 2 * p + 2)
                        nc.tensor.matmul(
                            ps[:], w[:, 0, sl, :], xs_hi[:, sl, nsl],
                            start=(p == 0), stop=False, perf_mode=DR,
                        )
                        nc.tensor.matmul(
                            ps[:], w[:, 0, sl, :], xs_lo[:, sl, nsl],
                            start=False, stop=False, perf_mode=DR,
                        )
                        nc.tensor.matmul(
                            ps[:], w[:, 1, sl, :], xs_hi[:, sl, nsl],
                            start=False, stop=(p == KC // 2 - 1), perf_mode=DR,
                        )
                        yield

                def rope_ops(m, n, ps):
                    """The four RoPE ops for one (feature tile, strip) pair."""
                    dst = qrot[m] if m < HQ else krot[m - HQ]
                    nsl = bass.ts(n, 512)
                    pt = ta.tile([128, 512], F32, name="pt")
                    p2 = psP2.tile([128, 512], F32, name="p2")
                    yield nc.vector.tensor_tensor(
                        pt[:], ps[:], trigf_sb[:, nsl], ALU.mult
                    )
                    yield nc.vector.tensor_tensor(
                        p2[:], ps[:], trigw_sb[:, nsl], ALU.mult
                    )
                    yield nc.vector.tensor_tensor(
                        dst[0:64, nsl], pt[0:64, :], p2[64:128, :], ALU.subtract
                    )
                    yield nc.vector.tensor_tensor(
                        dst[64:128, nsl], p2[0:64, :], pt[64:128, :], ALU.add
                    )

                def a_stream(m, pool):
                    w = load_wm(m)
                    yield
                    for n in range(NQ):
                        nsl = bass.ts(n, 512)
                        ps = pool.tile([128, 512], F32, name="psA")
                        yield from proj_mms(ps[:], w, nsl)
                        for _ in rope_ops(m, n, ps):
                            yield

                # ---- A0: v projection + k heads + q head 0 (pure PE phase) ----
                with tc.tile_pool(name="wvp", bufs=1) as wvp:
                    wv_sbh = wvp.tile([128, KC, HKV * HD], FP8, name="wvh")
                    wv_sbl = wvp.tile([128, KC, HKV * HD], FP8, name="wvl")
                    wk0 = load_wm(HQ)
                    for i in range(4):
                        ksl = slice(4 * i, 4 * i + 4)
                        nc.sync.dma_start(xs_hi[:, ksl, bass.ts(0, 512)],
                                          xhi[:, ksl, bass.ts(0, 512)])
                        nc.sync.dma_start(xs_lo[:, ksl, bass.ts(0, 512)],
                                          xlo[:, ksl, bass.ts(0, 512)])
                        if i == 0:
                            wk1 = load_wm(HQ + 1)
                        elif i == 1:
                            wq0 = load_wm(0)
                    nc.sync.dma_start(
                        wv_sbh[:], wv_hi.rearrange("p (kc c) -> p kc c", kc=KC)
                    )
                    nc.sync.dma_start(
                        wv_sbl[:], wv_lo.rearrange("p (kc c) -> p kc c", kc=KC)
                    )
                    nc.sync.dma_start(trigf_sb[:], trigf[:])
                    nc.sync.dma_start(trigw_sb[:], trigw[:])
                    nc.sync.dma_start(mask_sb[:], maskd[:])
                    nc.vector.memset(ones_mat[:], 1.0)
                    for n in range(NQ):
                        nsl = bass.ts(n, 512)
                        for m, w in ((HQ, wk0), (HQ + 1, wk1), (0, wq0)):
                            ps = psY.tile([128, 512], F32, name="psy")
                            for _ in proj_mms(ps[:], w, nsl):
                                pass
                            for _ in rope_ops(m, n, ps):
                                pass
                        if n + 1 < NQ:
                            nsl_next = bass.ts(n + 1, 512)
                            for i in range(4):
                                ksl = slice(4 * i, 4 * i + 4)
                                nc.sync.dma_start(xs_hi[:, ksl, nsl_next],
                                                  xhi[:, ksl, nsl_next])
                                nc.sync.dma_start(xs_lo[:, ksl, nsl_next],
                                                  xlo[:, ksl, nsl_next])
                        for tt in range(4 * n, 4 * n + 4):
                            # reuse the attention-phase psum slots during A0
                            tsl = slice(tt * 128, (tt + 1) * 128)
                            psv = psS.tile([128, 512], F32, name="pss")[
                                :, : HKV * HD
                            ]
                            for p in range(KC // 2):
                                sl = slice(2 * p, 2 * p + 2)
                                nc.tensor.matmul(
                                    psv[:], xs_hi[:, sl, tsl], wv_sbh[:, sl, :],
                                    start=(p == 0), stop=False, perf_mode=DR,
                                )
                                nc.tensor.matmul(
                                    psv[:], xs_hi[:, sl, tsl], wv_sbl[:, sl, :],
                                    start=False, stop=False, perf_mode=DR,
                                )
                                nc.tensor.matmul(
                                    psv[:], xs_lo[:, sl, tsl], wv_sbh[:, sl, :],
                                    start=False, stop=(p == KC // 2 - 1),
                                    perf_mode=DR,
                                )
                            nc.scalar.activation(
                                v_sb[:, tt, :], psv[:], AF.Copy, scale=1.0 / WS
                            )

                # ---- heads 0..6: attention + next head's projection ----
                for h in range(HQ - 1):
                    agen = a_stream(h + 1, psA)

                    def pop(kt, agen=agen):
                        next(agen, None)
                        if kt >= 10:
                            next(agen, None)

                    for qj in range(NQ):
                        ps_y, ps_d = emit_qj(h, qj, pop)
                        if state["pending"] is not None:
                            finalize(*state["pending"])
                        state["pending"] = (h, qj, ps_y, ps_d)
                    for _ in agen:
                        pass

            # ---- head 7: attention + output projection interleaved ----
            with (
                tc.tile_pool(name="wpc", bufs=4) as wpc,
                tc.tile_pool(name="obp", bufs=3) as obp,
                tc.tile_pool(name="psO", bufs=2, space="PSUM") as psO,
            ):
                def load_wp(fm):
                    wc = wpc.tile([128, 2, HQ, 128], FP8, name="wc")
                    nc.sync.dma_start(
                        wc[:], wp_hl[fm].rearrange("p (l h c) -> p l h c", l=2, h=HQ)
                    )
                    return wc

                def c_stream(n):
                    """Output projection for token strip n (16 feature tiles)."""
                    nsl = bass.ts(n, 512)
                    wcs = [load_wp(0), load_wp(1)]
                    for fm in range(FM):
                        wc = wcs.pop(0)
                        yield
                        ps_o = psO.tile([128, 512], F32, name="pso")
                        for p in range(HQ // 2):
                            sl = slice(2 * p, 2 * p + 2)
                            nc.tensor.matmul(
                                ps_o[:], wc[:, 0, sl, :], yt_hi[:, sl, nsl],
                                start=(p == 0), stop=False, perf_mode=DR,
                            )
                            nc.tensor.matmul(
                                ps_o[:], wc[:, 0, sl, :], yt_lo[:, sl, nsl],
                                start=False, stop=False, perf_mode=DR,
                            )
                            nc.tensor.matmul(
                                ps_o[:], wc[:, 1, sl, :], yt_hi[:, sl, nsl],
                                start=False, stop=(p == HQ // 2 - 1),
                                perf_mode=DR,
                            )
                            if p == 0 and fm + 2 < FM:
                                wcs.append(load_wp(fm + 2))
                            yield
                        ob = obp.tile([128, 512], BF16, name="ob")
                        nc.scalar.activation(ob[:], ps_o[:], AF.Copy, scale=1.0 / WS)
                        nc.sync.dma_start(
                            outT[fm * 128 : (fm + 1) * 128, nsl], ob[:]
                        )
                        yield

                cgens = []

                _end = object()

                def pop7(kt):
                    for _ in range(2):
                        while cgens:
                            if next(cgens[0], _end) is _end:
                                cgens.pop(0)
                                continue
                            break

                for qj in range(NQ):
                    ps_y, ps_d = emit_qj(HQ - 1, qj, pop7)
                    flush_ones()
                    if state["pending"] is not None:
                        finalize(*state["pending"])
                        state["pending"] = None
                    finalize(HQ - 1, qj, ps_y, ps_d)
                    cgens.append(c_stream(qj))
                # drain remaining output projection
                for g in cgens:
                    for _ in g:
                        pass

    nc.compile()
    return nc


def _get_nc():
    global _NC
    if _NC is None:
        _NC = build_nc()
    return _NC


def _split_hilo(a):
    """a = hi + lo with both parts e4m3."""
    hi = a.astype(E4)
    lo = (a - hi.astype(np.float32)).astype(E4)
    return hi, lo


def _prep_inputs(x, w_attn, w_proj):
    """Build the 8 per-core input maps from the full-problem arrays."""
    perm = np.concatenate([np.arange(0, HD, 2), np.arange(1, HD, 2)])

    f = np.arange(64, dtype=np.float64)
    inv = ROPE_THETA ** (-2.0 * f / HD)
    ang = inv[:, None] * np.arange(T, dtype=np.float64)[None, :]
    trigc = (np.cos(ang) / WS).astype(np.float32)
    trigs = (np.sin(ang) / WS).astype(np.float32)
    trigf = np.ascontiguousarray(np.concatenate([trigc, trigc], axis=0))
    trigw = np.ascontiguousarray(np.concatenate([trigs, trigs], axis=0))

    kk = np.arange(128)[:, None]
    qq = np.arange(128)[None, :]
    maskd = (kk <= qq).astype(BF)  # [128 k, 128 q] lower-triangle-valid

    w_attn = np.asarray(w_attn)
    w_proj = np.asarray(w_proj)
    x = np.asarray(x)

    in_maps = []
    for core in range(N_CORES):
        b, g = core // TP, core % TP
        # x features chunked: xhi[p, kc, t] = x[b].T[kc*128+p, t]
        xT = np.ascontiguousarray(x[b].T)  # [C, T] f32
        x_hi, x_lo = _split_hilo(xT)
        xhi = np.ascontiguousarray(x_hi.reshape(KC, 128, T).transpose(1, 0, 2))
        xlo = np.ascontiguousarray(x_lo.reshape(KC, 128, T).transpose(1, 0, 2))

        qrows = []
        for h in range(HQ):
            gh = g * HQ + h
            qrows.append(gh * HD + perm)
        for kv in range(HKV):
            gk = g * HKV + kv
            qrows.append(N_HEAD * HD + gk * HD + perm)
        qrows = np.concatenate(qrows)
        wqk = w_attn[qrows] * WS  # [1280, C] f32
        # stationary layout: wqk3[m, p, kc*128+col] = wqk[m*128+col, kc*128+p]
        wqk3 = np.ascontiguousarray(
            wqk.reshape(MQK, 128, KC, 128).transpose(0, 3, 2, 1).reshape(MQK, 128, C)
        )
        wqk3_hi, wqk3_lo = _split_hilo(wqk3)
        wqk_hl = np.ascontiguousarray(
            np.stack([wqk3_hi, wqk3_lo], axis=2).reshape(MQK, 128, 2 * C)
        )

        vrows = np.concatenate(
            [
                (N_HEAD + N_KV_HEAD) * HD + (g * HKV + kv) * HD + np.arange(HD)
                for kv in range(HKV)
            ]
        )
        wv = w_attn[vrows] * WS  # [256, C]
        # wv3[p, kc*256+c] = wv[c, kc*128+p]
        wv3 = np.ascontiguousarray(
            wv.reshape(HKV * HD, KC, 128).transpose(2, 1, 0).reshape(128, KC * HKV * HD)
        )
        wv3_hi, wv3_lo = _split_hilo(wv3)

        cols = np.arange(g * HQ * HD, (g + 1) * HQ * HD)
        wpg = w_proj[:, cols] * WS  # [C, 1024], rows = out features
        # wp5[fm, d, h*128+p] = wpg[fm*128+p, h*128+d]
        wp5 = np.ascontiguousarray(
            wpg.T.reshape(HQ, 128, FM, 128).transpose(2, 1, 0, 3).reshape(FM, 128, HQ * 128)
        )
        wp5_hi, wp5_lo = _split_hilo(wp5)
        wp_hl = np.ascontiguousarray(
            np.stack([wp5_hi, wp5_lo], axis=2).reshape(FM, 128, 2 * HQ * 128)
        )

        in_maps.append(
            {
                "xhi": xhi,
                "xlo": xlo,
                "wqk_hl": wqk_hl,
                "wv_hi": wv3_hi,
                "wv_lo": wv3_lo,
                "wp_hl": wp_hl,
                "trigf": trigf,
                "trigw": trigw,
                "maskd": maskd,
            }
        )
    return in_maps


def kernel(x, w_attn, w_proj):
    global LAST_RUN
    nc = _get_nc()
    in_maps = _prep_inputs(x, w_attn, w_proj)
    res = run_bass_kernel_spmd(nc, in_maps, core_ids=list(range(N_CORES)))
    LAST_RUN = res
    out = np.empty((B, T, C), dtype=np.float32)
    for b in range(B):
        acc = (
            res.results[TP * b]["outT"].astype(np.float32)
            + res.results[TP * b + 1]["outT"].astype(np.float32)
        )
        out[b] = acc.T
    return out


# revision 32
# speedup vs baseline: 1.2153x; 1.0094x over previous
"""Causal self-attention (GQA + RoPE) Trainium2 kernel, 8-way sharded.

Sharding: DP=4 over batch x TP=2 over kv-head groups (2 kv heads + their
8 q heads per group).  Each core computes its batch's qkv projection for
its head group, causal attention, and a partial c_proj (columns of
w_proj for its head group).  Host sums the two partial c_proj outputs
per batch.

Everything on-chip runs transposed ([feature, token] layout) so matmuls
contract along partitions; host transposes inputs/outputs.

Projection matmuls (qkv, v, c_proj) run as fp8e4 DoubleRow "triple-MMs":
each operand A is sent as A_hi + A_lo (both e4m3; hi = rounded value,
lo = rounded residual), and each pair of 128-deep contraction chunks is
computed with three DoubleRow matmuls
    hi*hi + hi*lo + lo*hi          (lo*lo ~ 0.06% -- dropped)
at half-rate each, i.e. 0.75x the bf16 cost with ~bf16 accuracy.
Weights are prescaled by 64 so their sigma ~ 1/45 lands in e4m3 normal
range; the inverse scale is folded into the RoPE trig tables / the
PSUM->SBUF copies.  Attention itself (QK, AV, softmax) stays bf16.

Pipeline: the attention inner loop leaves PE slack while ACT churns
exps, so the q/k projection work for head h+1 is interleaved into the
PE stream of head h's attention; head 7 interleaves c_proj instead.
The AV/denominator matmuls for tile kt are emitted two k-tiles late
(lag-2 software pipeline) so the exp -> mask chain never stalls the PE.
Causal masking only touches the one 128x128 triangle block per diagonal
tile; the valid column range of a diagonal tile is computed mask-free.

RoPE: w_attn q/k rows are permuted per-head to [even dims; odd dims] so
rotation pairs land at partition f and f+64 of the qkv psum tile:
  P  = ps * [c; c] (SBUF),  P2 = ps * [s; s] (PSUM)
  out[0:64]   = P[0:64]  - P2[64:128]
  out[64:128] = P2[0:64] + P[64:128]
(each combine reads one SBUF + one PSUM operand, which may sit at
different base partitions; two SBUF operands may not).

Softmax: att^T tiles ([k, q] layout) are exp'd on ACT without
max-subtraction (logits are O(6), fp32-safe).  Denominators: groups of
4 e-tiles are tree-summed on DVE and hit with one ones-column matmul
per group (deferred into the next group's PE stream); the per-q
reciprocal is broadcast down partitions with a f32r outer-product
matmul, also deferred one q-tile.
"""

import math

import numpy as np
import ml_dtypes

import concourse.bass as bass
import concourse.mybir as mybir
import concourse.tile as tile
from concourse import bacc
from concourse.bass_utils import run_bass_kernel_spmd

ALU = mybir.AluOpType
AF = mybir.ActivationFunctionType
F32 = mybir.dt.float32
F32R = mybir.dt.float32r
BF16 = mybir.dt.bfloat16
FP8 = mybir.dt.float8e4
DR = mybir.MatmulPerfMode.DoubleRow
BF = ml_dtypes.bfloat16
E4 = ml_dtypes.float8_e4m3

# problem shape (hardcoded per contest rules)
B, T, C = 4, 2048, 2048
N_HEAD, N_KV_HEAD, HD = 16, 4, 128
ROPE_THETA = 10000.0

TP = 2            # head-group shards
DP = 4            # batch shards
HQ = N_HEAD // TP         # 8 q heads per core
HKV = N_KV_HEAD // TP     # 2 kv heads per core
NREP = N_HEAD // N_KV_HEAD  # 4
QK_ROWS = (HQ + HKV) * HD   # 1280
KC = C // 128     # 16 contraction tiles
NQ = T // 512     # 4 token strips
MQK = QK_ROWS // 128  # 10 feature tiles (8 q heads + 2 kv heads)
FM = C // 128     # 16 output feature tiles
SCALE = 1.0 / math.sqrt(HD)
WS = 64.0         # weight prescale for e4m3 range

N_CORES = 8

_NC = None        # cached compiled Bass module
LAST_RUN = None   # BassKernelResults of the most recent kernel() call


def build_nc():
    nc = bacc.Bacc(None, target_bir_lowering=False, debug=False)

    xhi = nc.declare_dram_parameter("xhi", [128, KC, T], FP8, isOutput=False)
    xlo = nc.declare_dram_parameter("xlo", [128, KC, T], FP8, isOutput=False)
    wqk_hl = nc.declare_dram_parameter("wqk_hl", [MQK, 128, 2 * KC * 128], FP8, isOutput=False)
    wv_hi = nc.declare_dram_parameter("wv_hi", [128, KC * HKV * HD], FP8, isOutput=False)
    wv_lo = nc.declare_dram_parameter("wv_lo", [128, KC * HKV * HD], FP8, isOutput=False)
    wp_hl = nc.declare_dram_parameter("wp_hl", [FM, 128, 2 * HQ * 128], FP8, isOutput=False)
    trigf = nc.declare_dram_parameter("trigf", [128, T], F32, isOutput=False)  # [c;c]/WS
    trigw = nc.declare_dram_parameter("trigw", [128, T], F32, isOutput=False)  # [s;s]/WS
    maskd = nc.declare_dram_parameter("maskd", [128, 128], BF16, isOutput=False)
    outT = nc.declare_dram_parameter("outT", [C, T], BF16, isOutput=True)

    with tile.TileContext(nc) as tc:
        with (
            tc.tile_pool(name="const", bufs=1) as const,
            tc.tile_pool(name="persist", bufs=1) as persist,
            tc.tile_pool(name="eb", bufs=10) as eb,
            tc.tile_pool(name="gag", bufs=2) as gag,
            tc.tile_pool(name="rb", bufs=1) as rb,
            tc.tile_pool(name="ytp", bufs=3) as ytp,
            tc.tile_pool(name="psS", bufs=2, space="PSUM") as psS,
            tc.tile_pool(name="psY", bufs=2, space="PSUM") as psY,
            tc.tile_pool(name="psD", bufs=2, space="PSUM") as psD,
        ):
            trigf_sb = const.tile([128, T], F32, name="trigf")
            trigw_sb = const.tile([128, T], F32, name="trigw")
            mask_sb = const.tile([128, 128], BF16, name="mask")
            ones_mat = const.tile([128, 128], BF16, name="onem")

            qrot = [persist.tile([128, T], BF16, name=f"qrot{h}") for h in range(HQ)]
            krot = [persist.tile([128, T], BF16, name=f"krot{h}") for h in range(HKV)]
            v_sb = persist.tile([128, T // 128, HKV * HD], BF16, name="vtok")
            yt_hi = persist.tile([128, HQ, T], FP8, name="ythi")
            yt_lo = persist.tile([128, HQ, T], FP8, name="ytlo")

            state = {"pending": None, "pending_ones": None}

            def finalize(h, qj, ps_y, ps_d):
                qsl = bass.ts(qj, 512)
                r_sb = rb.tile([128, 512], F32, name="r")
                nc.vector.reciprocal(r_sb[:], ps_d[:])
                tmp = ytp.tile([128, 512], BF16, name="ytmp")
                nc.vector.tensor_tensor(tmp[:], ps_y[:], r_sb[:], ALU.mult)
                nc.gpsimd.tensor_copy(yt_hi[:, h, qsl], tmp[:])
                nc.gpsimd.tensor_tensor(
                    yt_lo[:, h, qsl], tmp[:], yt_hi[:, h, qsl], ALU.subtract
                )

            def flush_ones():
                if state["pending_ones"] is not None:
                    po, st, sp, pd = state["pending_ones"]
                    nc.tensor.matmul(pd[:], ones_mat[:], po[:], start=st, stop=sp)
                    state["pending_ones"] = None

            def strip_tiles(h, qj, res):
                """Generator: attention for (h, qj), yielding once per k-tile.

                AV/denominator matmuls run two k-tiles behind QK/exp (lag-2
                software pipeline).  The (ps_y, ps_d) accumulators are stored
                in res[qj] (finalized by the caller).
                """
                kvh = h // NREP
                vs = lambda kt: v_sb[:, kt, kvh * HD : (kvh + 1) * HD]
                ps_y = psY.tile([128, 512], F32, name="psy")
                ps_d = psD.tile([128, 512], F32, name="psd")
                res[qj] = (ps_y, ps_d)
                nkt = 4 * qj + 4
                g0 = ga = g2 = None
                hist = []

                def emit_av(e, d, lo, kt):
                    first = kt == 0
                    last = kt == nkt - 1
                    mid = lo + 128
                    if d >= 0 and mid < 512:
                        # valid columns first (no mask dependency), then the
                        # masked 128-wide triangle block
                        nc.tensor.matmul(
                            ps_y[:, mid:512], vs(kt), e[:, mid:512],
                            start=first, stop=False,
                        )
                        nc.tensor.matmul(
                            ps_y[:, lo:mid], vs(kt), e[:, lo:mid],
                            start=False, stop=last,
                        )
                    else:
                        nc.tensor.matmul(
                            ps_y[:, lo:512], vs(kt), e[:, lo:512],
                            start=first, stop=last,
                        )
                    if d >= 0:
                        if d == 0:
                            flush_ones()
                        nc.tensor.matmul(
                            ps_d[:, lo:512], ones_mat[:], e[:, lo:512],
                            start=(qj == 0 and kt == 0), stop=last,
                        )

                for kt in range(nkt):
                    d = kt - 4 * qj
                    # diagonal tile d has valid q-columns only in [128d, 512)
                    lo = 128 * d if d > 0 else 0
                    qlo = qj * 512 + lo
                    ps_s = psS.tile([128, 512], F32, name="pss")
                    nc.tensor.matmul(
                        ps_s[:, lo:512],
                        krot[kvh][:, kt * 128 : (kt + 1) * 128],
                        qrot[h][:, qlo : (qj + 1) * 512],
                        start=True,
                        stop=True,
                    )
                    e = eb.tile([128, 512], BF16, name="e")
                    nc.scalar.activation(
                        e[:, lo:512], ps_s[:, lo:512], AF.Exp, scale=SCALE
                    )
                    if d >= 0:
                        # mask only the 128x128 triangle block (on GPSIMD --
                        # the lag-2 AV emission gives the chain plenty of slack)
                        nc.gpsimd.tensor_tensor(
                            e[:, lo : lo + 128], e[:, lo : lo + 128],
                            mask_sb[:], ALU.mult,
                        )
                    else:
                        # full groups: tree-sum 4 e-tiles (first add on GPSIMD,
                        # rest on DVE), one deferred ones-matmul per group
                        # (emitted in a later PE slot so the PE never waits on
                        # the adds).
                        ph = kt % 4
                        if ph == 0:
                            g0 = e
                        elif ph == 1:
                            ga = gag.tile([128, 512], BF16, name="ga")
                            nc.gpsimd.tensor_tensor(ga[:], g0[:], e[:], ALU.add)
                        elif ph == 2:
                            g2 = e
                        else:
                            gs = gag.tile([128, 512], BF16, name="gs")
                            nc.vector.tensor_tensor(gs[:], g2[:], e[:], ALU.add)
                            nc.vector.tensor_tensor(gs[:], gs[:], ga[:], ALU.add)
                            flush_ones()
                            grp = kt // 4
                            state["pending_ones"] = (gs, grp == 0, False, ps_d)
                    hist.append((e, d, lo, kt))
                    if len(hist) > 2:
                        emit_av(*hist.pop(0))
                    yield
                for item in hist:
                    emit_av(*item)

            def emit_qj(h, qj, pop):
                """Attention for (h, qj), with pop() called once per k-tile."""
                res = {}
                gen = strip_tiles(h, qj, res)
                kt = 0
                while next(gen, _END) is not _END:
                    pop(kt)
                    kt += 1
                return res[qj]

            _END = object()

            # ======== projection machinery (lives through heads 0..6) ========
            with (
                tc.tile_pool(name="xa", bufs=1) as xa,
                tc.tile_pool(name="wm", bufs=4) as wm,
                tc.tile_pool(name="ta", bufs=1) as ta,
                tc.tile_pool(name="psA", bufs=1, space="PSUM") as psA,
                tc.tile_pool(name="psP2", bufs=1, space="PSUM") as psP2,
            ):
                xs_hi = xa.tile([128, KC, T], FP8, name="xshi")
                xs_lo = xa.tile([128, KC, T], FP8, name="xslo")

                def load_wm(m):
                    w = wm.tile([128, 2, KC, 128], FP8, name="wmhl")
                    nc.sync.dma_start(
                        w[:], wqk_hl[m].rearrange("p (l kc c) -> p l kc c", l=2, kc=KC)
                    )
                    return w

                def proj_mms(ps, w, nsl):
                    """Triple-MM qkv projection chunk stream for one strip."""
                    for p in range(KC // 2):
                        sl = slice(2 * p, 2 * p + 2)
                        nc.tensor.matmul(
                            ps[:], w[:, 0, sl, :], xs_hi[:, sl, nsl],
                            start=(p == 0), stop=False, perf_mode=DR,
                        )
                        nc.tensor.matmul(
                            ps[:], w[:, 0, sl, :], xs_lo[:, sl, nsl],
                            start=False, stop=False, perf_mode=DR,
                        )
                        nc.tensor.matmul(
                            ps[:], w[:, 1, sl, :], xs_hi[:, sl, nsl],
                            start=False, stop=(p == KC // 2 - 1), perf_mode=DR,
                        )
                        yield

                def rope_ops(m, n, ps):
                    """The four RoPE ops for one (feature tile, strip) pair."""
                    dst = qrot[m] if m < HQ else krot[m - HQ]
                    nsl = bass.ts(n, 512)
                    pt = ta.tile([128, 512], F32, name="pt")
                    p2 = psP2.tile([128, 512], F32, name="p2")
                    yield nc.vector.tensor_tensor(
                        pt[:], ps[:], trigf_sb[:, nsl], ALU.mult
                    )
                    yield nc.vector.tensor_tensor(
                        p2[:], ps[:], trigw_sb[:, nsl], ALU.mult
                    )
                    yield nc.vector.tensor_tensor(
                        dst[0:64, nsl], pt[0:64, :], p2[64:128, :], ALU.subtract
                    )
                    yield nc.vector.tensor_tensor(
                        dst[64:128, nsl], p2[0:64, :], pt[64:128, :], ALU.add
                    )

                def a_stream(m, pool):
                    w = load_wm(m)
                    yield
                    for n in range(NQ):
                        nsl = bass.ts(n, 512)
                        ps = pool.tile([128, 512], F32, name="psA")
                        yield from proj_mms(ps[:], w, nsl)
                        for _ in rope_ops(m, n, ps):
                            yield

                # ---- A0: v projection + k heads + q head 0 (pure PE phase) ----
                with tc.tile_pool(name="wvp", bufs=1) as wvp:
                    wv_sbh = wvp.tile([128, KC, HKV * HD], FP8, name="wvh")
                    wv_sbl = wvp.tile([128, KC, HKV * HD], FP8, name="wvl")
                    wk0 = load_wm(HQ)
                    for i in range(4):
                        ksl = slice(4 * i, 4 * i + 4)
                        nc.sync.dma_start(xs_hi[:, ksl, bass.ts(0, 512)],
                                          xhi[:, ksl, bass.ts(0, 512)])
                        nc.sync.dma_start(xs_lo[:, ksl, bass.ts(0, 512)],
                                          xlo[:, ksl, bass.ts(0, 512)])
                        if i == 0:
                            wk1 = load_wm(HQ + 1)
                        elif i == 1:
                            wq0 = load_wm(0)
                    nc.sync.dma_start(
                        wv_sbh[:], wv_hi.rearrange("p (kc c) -> p kc c", kc=KC)
                    )
                    nc.sync.dma_start(
                        wv_sbl[:], wv_lo.rearrange("p (kc c) -> p kc c", kc=KC)
                    )
                    nc.sync.dma_start(trigf_sb[:], trigf[:])
                    nc.sync.dma_start(trigw_sb[:], trigw[:])
                    nc.sync.dma_start(mask_sb[:], maskd[:])
                    nc.vector.memset(ones_mat[:], 1.0)
                    for n in range(NQ):
                        nsl = bass.ts(n, 512)
                        for m, w in ((HQ, wk0), (HQ + 1, wk1), (0, wq0)):
                            ps = psY.tile([128, 512], F32, name="psy")
                            for _ in proj_mms(ps[:], w, nsl):
                                pass
                            for _ in rope_ops(m, n, ps):
                                pass
                        if n + 1 < NQ:
                            nsl_next = bass.ts(n + 1, 512)
                            for i in range(4):
                                ksl = slice(4 * i, 4 * i + 4)
                                nc.sync.dma_start(xs_hi[:, ksl, nsl_next],
                                                  xhi[:, ksl, nsl_next])
                                nc.sync.dma_start(xs_lo[:, ksl, nsl_next],
                                                  xlo[:, ksl, nsl_next])
                        for tt in range(4 * n, 4 * n + 4):
                            # reuse the attention-phase psum slots during A0
                            tsl = slice(tt * 128, (tt + 1) * 128)
                            psv = psS.tile([128, 512], F32, name="pss")[
                                :, : HKV * HD
                            ]
                            for p in range(KC // 2):
                                sl = slice(2 * p, 2 * p + 2)
                                nc.tensor.matmul(
                                    psv[:], xs_hi[:, sl, tsl], wv_sbh[:, sl, :],
                                    start=(p == 0), stop=False, perf_mode=DR,
                                )
                                nc.tensor.matmul(
                                    psv[:], xs_hi[:, sl, tsl], wv_sbl[:, sl, :],
                                    start=False, stop=False, perf_mode=DR,
                                )
                                nc.tensor.matmul(
                                    psv[:], xs_lo[:, sl, tsl], wv_sbh[:, sl, :],
                                    start=False, stop=(p == KC // 2 - 1),
                                    perf_mode=DR,
                                )
                            nc.scalar.activation(
                                v_sb[:, tt, :], psv[:], AF.Copy, scale=1.0 / WS
                            )

                # ---- heads 0..6: attention + next head's projection ----
                for h in range(HQ - 1):
                    agen = a_stream(h + 1, psA)

                    def pop(kt, agen=agen):
                        next(agen, None)
                        if kt >= 10:
                            next(agen, None)

                    for qj in range(NQ):
                        ps_y, ps_d = emit_qj(h, qj, pop)
                        finalize(h, qj, ps_y, ps_d)
                    for _ in agen:
                        pass

            # ---- head 7: attention + output projection interleaved ----
            with (
                tc.tile_pool(name="wpc", bufs=6) as wpc,
                tc.tile_pool(name="obp", bufs=3) as obp,
                tc.tile_pool(name="psO", bufs=2, space="PSUM") as psO,
            ):
                def load_wp(fm):
                    wc = wpc.tile([128, 2, HQ, 128], FP8, name="wc")
                    nc.sync.dma_start(
                        wc[:], wp_hl[fm].rearrange("p (l h c) -> p l h c", l=2, h=HQ)
                    )
                    return wc

                def c_stream(n):
                    """Output projection for token strip n (16 feature tiles)."""
                    nsl = bass.ts(n, 512)
                    wcs = [load_wp(0), load_wp(1), load_wp(2)]
                    for fm in range(FM):
                        wc = wcs.pop(0)
                        yield
                        ps_o = psO.tile([128, 512], F32, name="pso")
                        for p in range(HQ // 2):
                            sl = slice(2 * p, 2 * p + 2)
                            nc.tensor.matmul(
                                ps_o[:], wc[:, 0, sl, :], yt_hi[:, sl, nsl],
                                start=(p == 0), stop=False, perf_mode=DR,
                            )
                            nc.tensor.matmul(
                                ps_o[:], wc[:, 0, sl, :], yt_lo[:, sl, nsl],
                                start=False, stop=False, perf_mode=DR,
                            )
                            nc.tensor.matmul(
                                ps_o[:], wc[:, 1, sl, :], yt_hi[:, sl, nsl],
                                start=False, stop=(p == HQ // 2 - 1),
                                perf_mode=DR,
                            )
                            if p == 0 and fm + 3 < FM:
                                wcs.append(load_wp(fm + 3))
                            yield
                        ob = obp.tile([128, 512], BF16, name="ob")
                        nc.scalar.activation(ob[:], ps_o[:], AF.Copy, scale=1.0 / WS)
                        nc.sync.dma_start(
                            outT[fm * 128 : (fm + 1) * 128, nsl], ob[:]
                        )
                        yield

                cgens = []

                _end = object()

                def pop7(kt):
                    for _ in range(6):
                        while cgens:
                            if next(cgens[0], _end) is _end:
                                cgens.pop(0)
                                continue
                            break

                for qj in range(NQ):
                    ps_y, ps_d = emit_qj(HQ - 1, qj, pop7)
                    flush_ones()
                    finalize(HQ - 1, qj, ps_y, ps_d)
                    cgens.append(c_stream(qj))
                # drain remaining output projection
                for g in cgens:
                    for _ in g:
                        pass

    nc.compile()
    return nc


def _get_nc():
    global _NC
    if _NC is None:
        _NC = build_nc()
    return _NC


def _split_hilo(a):
    """a = hi + lo with both parts e4m3."""
    hi = a.astype(E4)
    lo = (a - hi.astype(np.float32)).astype(E4)
    return hi, lo


def _prep_inputs(x, w_attn, w_proj):
    """Build the 8 per-core input maps from the full-problem arrays."""
    perm = np.concatenate([np.arange(0, HD, 2), np.arange(1, HD, 2)])

    f = np.arange(64, dtype=np.float64)
    inv = ROPE_THETA ** (-2.0 * f / HD)
    ang = inv[:, None] * np.arange(T, dtype=np.float64)[None, :]
    trigc = (np.cos(ang) / WS).astype(np.float32)
    trigs = (np.sin(ang) / WS).astype(np.float32)
    trigf = np.ascontiguousarray(np.concatenate([trigc, trigc], axis=0))
    trigw = np.ascontiguousarray(np.concatenate([trigs, trigs], axis=0))

    kk = np.arange(128)[:, None]
    qq = np.arange(128)[None, :]
    maskd = (kk <= qq).astype(BF)  # [128 k, 128 q] lower-triangle-valid

    w_attn = np.asarray(w_attn)
    w_proj = np.asarray(w_proj)
    x = np.asarray(x)

    in_maps = []
    for core in range(N_CORES):
        b, g = core // TP, core % TP
        # x features chunked: xhi[p, kc, t] = x[b].T[kc*128+p, t]
        xT = np.ascontiguousarray(x[b].T)  # [C, T] f32
        x_hi, x_lo = _split_hilo(xT)
        xhi = np.ascontiguousarray(x_hi.reshape(KC, 128, T).transpose(1, 0, 2))
        xlo = np.ascontiguousarray(x_lo.reshape(KC, 128, T).transpose(1, 0, 2))

        qrows = []
        for h in range(HQ):
            gh = g * HQ + h
            qrows.append(gh * HD + perm)
        for kv in range(HKV):
            gk = g * HKV + kv
            qrows.append(N_HEAD * HD + gk * HD + perm)
        qrows = np.concatenate(qrows)
        wqk = w_attn[qrows] * WS  # [1280, C] f32
        # stationary layout: wqk3[m, p, kc*128+col] = wqk[m*128+col, kc*128+p]
        wqk3 = np.ascontiguousarray(
            wqk.reshape(MQK, 128, KC, 128).transpose(0, 3, 2, 1).reshape(MQK, 128, C)
        )
        wqk3_hi, wqk3_lo = _split_hilo(wqk3)
        wqk_hl = np.ascontiguousarray(
            np.stack([wqk3_hi, wqk3_lo], axis=2).reshape(MQK, 128, 2 * C)
        )

        vrows = np.concatenate(
            [
                (N_HEAD + N_KV_HEAD) * HD + (g * HKV + kv) * HD + np.arange(HD)
                for kv in range(HKV)
            ]
        )
        wv = w_attn[vrows] * WS  # [256, C]
        # wv3[p, kc*256+c] = wv[c, kc*128+p]
        wv3 = np.ascontiguousarray(
            wv.reshape(HKV * HD, KC, 128).transpose(2, 1, 0).reshape(128, KC * HKV * HD)
        )
        wv3_hi, wv3_lo = _split_hilo(wv3)

        cols = np.arange(g * HQ * HD, (g + 1) * HQ * HD)
        wpg = w_proj[:, cols] * WS  # [C, 1024], rows = out features
        # wp5[fm, d, h*128+p] = wpg[fm*128+p, h*128+d]
        wp5 = np.ascontiguousarray(
            wpg.T.reshape(HQ, 128, FM, 128).transpose(2, 1, 0, 3).reshape(FM, 128, HQ * 128)
        )
        wp5_hi, wp5_lo = _split_hilo(wp5)
        wp_hl = np.ascontiguousarray(
            np.stack([wp5_hi, wp5_lo], axis=2).reshape(FM, 128, 2 * HQ * 128)
        )

        in_maps.append(
            {
                "xhi": xhi,
                "xlo": xlo,
                "wqk_hl": wqk_hl,
                "wv_hi": wv3_hi,
                "wv_lo": wv3_lo,
                "wp_hl": wp_hl,
                "trigf": trigf,
                "trigw": trigw,
                "maskd": maskd,
            }
        )
    return in_maps


def kernel(x, w_attn, w_proj):
    global LAST_RUN
    nc = _get_nc()
    in_maps = _prep_inputs(x, w_attn, w_proj)
    res = run_bass_kernel_spmd(nc, in_maps, core_ids=list(range(N_CORES)))
    LAST_RUN = res
    out = np.empty((B, T, C), dtype=np.float32)
    for b in range(B):
        acc = (
            res.results[TP * b]["outT"].astype(np.float32)
            + res.results[TP * b + 1]["outT"].astype(np.float32)
        )
        out[b] = acc.T
    return out


# revision 33
# speedup vs baseline: 1.2174x; 1.0018x over previous
"""Causal self-attention (GQA + RoPE) Trainium2 kernel, 8-way sharded.

Sharding: DP=4 over batch x TP=2 over kv-head groups (2 kv heads + their
8 q heads per group).  Each core computes its batch's qkv projection for
its head group, causal attention, and a partial c_proj (columns of
w_proj for its head group).  Host sums the two partial c_proj outputs
per batch.

Everything on-chip runs transposed ([feature, token] layout) so matmuls
contract along partitions; host transposes inputs/outputs.

Projection matmuls (qkv, v, c_proj) run as fp8e4 DoubleRow "triple-MMs":
each operand A is sent as A_hi + A_lo (both e4m3; hi = rounded value,
lo = rounded residual), and each pair of 128-deep contraction chunks is
computed with three DoubleRow matmuls
    hi*hi + hi*lo + lo*hi          (lo*lo ~ 0.06% -- dropped)
at half-rate each, i.e. 0.75x the bf16 cost with ~bf16 accuracy.
Weights are prescaled by 64 so their sigma ~ 1/45 lands in e4m3 normal
range; the inverse scale is folded into the RoPE trig tables / the
PSUM->SBUF copies.  Attention itself (QK, AV, softmax) stays bf16.

Pipeline: the attention inner loop leaves PE slack while ACT churns
exps, so the q/k projection work for head h+1 is interleaved into the
PE stream of head h's attention; head 7 interleaves c_proj instead.
The AV/denominator matmuls for tile kt are emitted two k-tiles late
(lag-2 software pipeline) so the exp -> mask chain never stalls the PE.
Causal masking only touches the one 128x128 triangle block per diagonal
tile; the valid column range of a diagonal tile is computed mask-free.

RoPE: w_attn q/k rows are permuted per-head to [even dims; odd dims] so
rotation pairs land at partition f and f+64 of the qkv psum tile:
  P  = ps * [c; c] (SBUF),  P2 = ps * [s; s] (PSUM)
  out[0:64]   = P[0:64]  - P2[64:128]
  out[64:128] = P2[0:64] + P[64:128]
(each combine reads one SBUF + one PSUM operand, which may sit at
different base partitions; two SBUF operands may not).

Softmax: att^T tiles ([k, q] layout) are exp'd on ACT without
max-subtraction (logits are O(6), fp32-safe).  Denominators: groups of
4 e-tiles are tree-summed on DVE and hit with one ones-MATRIX matmul
per group (deferred into the next group's PE stream) whose [128, 512]
output IS the denominator broadcast down all partitions -- the per-q
reciprocal is then a single DVE op, no extra broadcast needed.
"""

import math

import numpy as np
import ml_dtypes

import concourse.bass as bass
import concourse.mybir as mybir
import concourse.tile as tile
from concourse import bacc
from concourse.bass_utils import run_bass_kernel_spmd

ALU = mybir.AluOpType
AF = mybir.ActivationFunctionType
F32 = mybir.dt.float32
F32R = mybir.dt.float32r
BF16 = mybir.dt.bfloat16
FP8 = mybir.dt.float8e4
DR = mybir.MatmulPerfMode.DoubleRow
BF = ml_dtypes.bfloat16
E4 = ml_dtypes.float8_e4m3

# problem shape (hardcoded per contest rules)
B, T, C = 4, 2048, 2048
N_HEAD, N_KV_HEAD, HD = 16, 4, 128
ROPE_THETA = 10000.0

TP = 2            # head-group shards
DP = 4            # batch shards
HQ = N_HEAD // TP         # 8 q heads per core
HKV = N_KV_HEAD // TP     # 2 kv heads per core
NREP = N_HEAD // N_KV_HEAD  # 4
QK_ROWS = (HQ + HKV) * HD   # 1280
KC = C // 128     # 16 contraction tiles
NQ = T // 512     # 4 token strips
MQK = QK_ROWS // 128  # 10 feature tiles (8 q heads + 2 kv heads)
FM = C // 128     # 16 output feature tiles
SCALE = 1.0 / math.sqrt(HD)
WS = 64.0         # weight prescale for e4m3 range

N_CORES = 8

_NC = None        # cached compiled Bass module
LAST_RUN = None   # BassKernelResults of the most recent kernel() call


def build_nc():
    nc = bacc.Bacc(None, target_bir_lowering=False, debug=False)

    xhi = nc.declare_dram_parameter("xhi", [128, KC, T], FP8, isOutput=False)
    xlo = nc.declare_dram_parameter("xlo", [128, KC, T], FP8, isOutput=False)
    wqk_hl = nc.declare_dram_parameter("wqk_hl", [MQK, 128, 2 * KC * 128], FP8, isOutput=False)
    wv_hi = nc.declare_dram_parameter("wv_hi", [128, KC * HKV * HD], FP8, isOutput=False)
    wv_lo = nc.declare_dram_parameter("wv_lo", [128, KC * HKV * HD], FP8, isOutput=False)
    wp_hl = nc.declare_dram_parameter("wp_hl", [FM, 128, 2 * HQ * 128], FP8, isOutput=False)
    trigf = nc.declare_dram_parameter("trigf", [128, T], F32, isOutput=False)  # [c;c]/WS
    trigw = nc.declare_dram_parameter("trigw", [128, T], F32, isOutput=False)  # [s;s]/WS
    maskd = nc.declare_dram_parameter("maskd", [128, 128], BF16, isOutput=False)
    outT = nc.declare_dram_parameter("outT", [C, T], BF16, isOutput=True)

    with tile.TileContext(nc) as tc:
        with (
            tc.tile_pool(name="const", bufs=1) as const,
            tc.tile_pool(name="persist", bufs=1) as persist,
            tc.tile_pool(name="eb", bufs=10) as eb,
            tc.tile_pool(name="gag", bufs=2) as gag,
            tc.tile_pool(name="rb", bufs=1) as rb,
            tc.tile_pool(name="ytp", bufs=3) as ytp,
            tc.tile_pool(name="psS", bufs=2, space="PSUM") as psS,
            tc.tile_pool(name="psY", bufs=2, space="PSUM") as psY,
            tc.tile_pool(name="psD", bufs=2, space="PSUM") as psD,
        ):
            trigf_sb = const.tile([128, T], F32, name="trigf")
            trigw_sb = const.tile([128, T], F32, name="trigw")
            mask_sb = const.tile([128, 128], BF16, name="mask")
            ones_mat = const.tile([128, 128], BF16, name="onem")

            qrot = [persist.tile([128, T], BF16, name=f"qrot{h}") for h in range(HQ)]
            krot = [persist.tile([128, T], BF16, name=f"krot{h}") for h in range(HKV)]
            v_sb = persist.tile([128, T // 128, HKV * HD], BF16, name="vtok")
            yt_hi = persist.tile([128, HQ, T], FP8, name="ythi")
            yt_lo = persist.tile([128, HQ, T], FP8, name="ytlo")

            state = {"pending": None, "pending_ones": None}

            def finalize(h, qj, ps_y, ps_d):
                qsl = bass.ts(qj, 512)
                r_sb = rb.tile([128, 512], F32, name="r")
                nc.vector.reciprocal(r_sb[:], ps_d[:])
                tmp = ytp.tile([128, 512], BF16, name="ytmp")
                nc.vector.tensor_tensor(tmp[:], ps_y[:], r_sb[:], ALU.mult)
                nc.gpsimd.tensor_copy(yt_hi[:, h, qsl], tmp[:])
                nc.gpsimd.tensor_tensor(
                    yt_lo[:, h, qsl], tmp[:], yt_hi[:, h, qsl], ALU.subtract
                )

            def flush_ones():
                if state["pending_ones"] is not None:
                    po, st, sp, pd = state["pending_ones"]
                    nc.tensor.matmul(pd[:], ones_mat[:], po[:], start=st, stop=sp)
                    state["pending_ones"] = None

            def strip_tiles(h, qj, res):
                """Generator: attention for (h, qj), yielding once per k-tile.

                AV/denominator matmuls run two k-tiles behind QK/exp (lag-2
                software pipeline).  The (ps_y, ps_d) accumulators are stored
                in res[qj] (finalized by the caller).
                """
                kvh = h // NREP
                vs = lambda kt: v_sb[:, kt, kvh * HD : (kvh + 1) * HD]
                ps_y = psY.tile([128, 512], F32, name="psy")
                ps_d = psD.tile([128, 512], F32, name="psd")
                res[qj] = (ps_y, ps_d)
                nkt = 4 * qj + 4
                g0 = ga = g2 = None
                hist = []

                def emit_av(e, d, lo, kt):
                    first = kt == 0
                    last = kt == nkt - 1
                    mid = lo + 128
                    if d >= 0 and mid < 512:
                        # valid columns first (no mask dependency), then the
                        # masked 128-wide triangle block
                        nc.tensor.matmul(
                            ps_y[:, mid:512], vs(kt), e[:, mid:512],
                            start=first, stop=False,
                        )
                        nc.tensor.matmul(
                            ps_y[:, lo:mid], vs(kt), e[:, lo:mid],
                            start=False, stop=last,
                        )
                    else:
                        nc.tensor.matmul(
                            ps_y[:, lo:512], vs(kt), e[:, lo:512],
                            start=first, stop=last,
                        )
                    if d >= 0:
                        if d == 0:
                            flush_ones()
                        nc.tensor.matmul(
                            ps_d[:, lo:512], ones_mat[:], e[:, lo:512],
                            start=(qj == 0 and kt == 0), stop=last,
                        )

                for kt in range(nkt):
                    d = kt - 4 * qj
                    # diagonal tile d has valid q-columns only in [128d, 512)
                    lo = 128 * d if d > 0 else 0
                    qlo = qj * 512 + lo
                    ps_s = psS.tile([128, 512], F32, name="pss")
                    nc.tensor.matmul(
                        ps_s[:, lo:512],
                        krot[kvh][:, kt * 128 : (kt + 1) * 128],
                        qrot[h][:, qlo : (qj + 1) * 512],
                        start=True,
                        stop=True,
                    )
                    e = eb.tile([128, 512], BF16, name="e")
                    nc.scalar.activation(
                        e[:, lo:512], ps_s[:, lo:512], AF.Exp, scale=SCALE
                    )
                    if d >= 0:
                        # mask only the 128x128 triangle block (on GPSIMD --
                        # the lag-2 AV emission gives the chain plenty of slack)
                        nc.gpsimd.tensor_tensor(
                            e[:, lo : lo + 128], e[:, lo : lo + 128],
                            mask_sb[:], ALU.mult,
                        )
                    else:
                        # full groups: tree-sum 4 e-tiles (first add on GPSIMD,
                        # rest on DVE), one deferred ones-matmul per group
                        # (emitted in a later PE slot so the PE never waits on
                        # the adds).
                        ph = kt % 4
                        if ph == 0:
                            g0 = e
                        elif ph == 1:
                            ga = gag.tile([128, 512], BF16, name="ga")
                            nc.gpsimd.tensor_tensor(ga[:], g0[:], e[:], ALU.add)
                        elif ph == 2:
                            g2 = e
                        else:
                            gs = gag.tile([128, 512], BF16, name="gs")
                            nc.vector.tensor_tensor(gs[:], g2[:], e[:], ALU.add)
                            nc.vector.tensor_tensor(gs[:], gs[:], ga[:], ALU.add)
                            flush_ones()
                            grp = kt // 4
                            state["pending_ones"] = (gs, grp == 0, False, ps_d)
                    hist.append((e, d, lo, kt))
                    if len(hist) > 2:
                        emit_av(*hist.pop(0))
                    yield
                for item in hist:
                    emit_av(*item)

            def emit_qj(h, qj, pop):
                """Attention for (h, qj), with pop() called once per k-tile."""
                res = {}
                gen = strip_tiles(h, qj, res)
                kt = 0
                while next(gen, _END) is not _END:
                    pop(kt)
                    kt += 1
                return res[qj]

            _END = object()

            # ======== projection machinery (lives through heads 0..6) ========
            with (
                tc.tile_pool(name="xa", bufs=1) as xa,
                tc.tile_pool(name="wm", bufs=4) as wm,
                tc.tile_pool(name="ta", bufs=1) as ta,
                tc.tile_pool(name="psA", bufs=1, space="PSUM") as psA,
                tc.tile_pool(name="psP2", bufs=1, space="PSUM") as psP2,
            ):
                xs_hi = xa.tile([128, KC, T], FP8, name="xshi")
                xs_lo = xa.tile([128, KC, T], FP8, name="xslo")

                def load_wm(m):
                    w = wm.tile([128, 2, KC, 128], FP8, name="wmhl")
                    nc.sync.dma_start(
                        w[:], wqk_hl[m].rearrange("p (l kc c) -> p l kc c", l=2, kc=KC)
                    )
                    return w

                def proj_mms(ps, w, nsl):
                    """Triple-MM qkv projection chunk stream for one strip."""
                    for p in range(KC // 2):
                        sl = slice(2 * p, 2 * p + 2)
                        nc.tensor.matmul(
                            ps[:], w[:, 0, sl, :], xs_hi[:, sl, nsl],
                            start=(p == 0), stop=False, perf_mode=DR,
                        )
                        nc.tensor.matmul(
                            ps[:], w[:, 0, sl, :], xs_lo[:, sl, nsl],
                            start=False, stop=False, perf_mode=DR,
                        )
                        nc.tensor.matmul(
                            ps[:], w[:, 1, sl, :], xs_hi[:, sl, nsl],
                            start=False, stop=(p == KC // 2 - 1), perf_mode=DR,
                        )
                        yield

                def rope_ops(m, n, ps):
                    """The four RoPE ops for one (feature tile, strip) pair."""
                    dst = qrot[m] if m < HQ else krot[m - HQ]
                    nsl = bass.ts(n, 512)
                    pt = ta.tile([128, 512], F32, name="pt")
                    p2 = psP2.tile([128, 512], F32, name="p2")
                    yield nc.vector.tensor_tensor(
                        pt[:], ps[:], trigf_sb[:, nsl], ALU.mult
                    )
                    yield nc.vector.tensor_tensor(
                        p2[:], ps[:], trigw_sb[:, nsl], ALU.mult
                    )
                    yield nc.vector.tensor_tensor(
                        dst[0:64, nsl], pt[0:64, :], p2[64:128, :], ALU.subtract
                    )
                    yield nc.vector.tensor_tensor(
                        dst[64:128, nsl], p2[0:64, :], pt[64:128, :], ALU.add
                    )

                def a_stream(m, pool):
                    w = load_wm(m)
                    yield
                    for n in range(NQ):
                        nsl = bass.ts(n, 512)
                        ps = pool.tile([128, 512], F32, name="psA")
                        yield from proj_mms(ps[:], w, nsl)
                        for _ in rope_ops(m, n, ps):
                            yield

                # ---- A0: v projection + k heads + q head 0 (pure PE phase) ----
                with tc.tile_pool(name="wvp", bufs=1) as wvp:
                    wv_sbh = wvp.tile([128, KC, HKV * HD], FP8, name="wvh")
                    wv_sbl = wvp.tile([128, KC, HKV * HD], FP8, name="wvl")
                    wk0 = load_wm(HQ)
                    for i in range(4):
                        ksl = slice(4 * i, 4 * i + 4)
                        nc.sync.dma_start(xs_hi[:, ksl, bass.ts(0, 512)],
                                          xhi[:, ksl, bass.ts(0, 512)])
                        nc.sync.dma_start(xs_lo[:, ksl, bass.ts(0, 512)],
                                          xlo[:, ksl, bass.ts(0, 512)])
                        if i == 0:
                            wk1 = load_wm(HQ + 1)
                        elif i == 1:
                            wq0 = load_wm(0)
                    nc.sync.dma_start(
                        wv_sbh[:], wv_hi.rearrange("p (kc c) -> p kc c", kc=KC)
                    )
                    nc.sync.dma_start(
                        wv_sbl[:], wv_lo.rearrange("p (kc c) -> p kc c", kc=KC)
                    )
                    nc.sync.dma_start(trigf_sb[:], trigf[:])
                    nc.sync.dma_start(trigw_sb[:], trigw[:])
                    nc.sync.dma_start(mask_sb[:], maskd[:])
                    nc.vector.memset(ones_mat[:], 1.0)
                    for n in range(NQ):
                        nsl = bass.ts(n, 512)
                        for m, w in ((HQ, wk0), (HQ + 1, wk1), (0, wq0)):
                            ps = psY.tile([128, 512], F32, name="psy")
                            for _ in proj_mms(ps[:], w, nsl):
                                pass
                            for _ in rope_ops(m, n, ps):
                                pass
                        if n + 1 < NQ:
                            nsl_next = bass.ts(n + 1, 512)
                            for i in range(4):
                                ksl = slice(4 * i, 4 * i + 4)
                                nc.sync.dma_start(xs_hi[:, ksl, nsl_next],
                                                  xhi[:, ksl, nsl_next])
                                nc.sync.dma_start(xs_lo[:, ksl, nsl_next],
                                                  xlo[:, ksl, nsl_next])
                        for tt in range(4 * n, 4 * n + 4):
                            # reuse the attention-phase psum slots during A0
                            tsl = slice(tt * 128, (tt + 1) * 128)
                            psv = psS.tile([128, 512], F32, name="pss")[
                                :, : HKV * HD
                            ]
                            for p in range(KC // 2):
                                sl = slice(2 * p, 2 * p + 2)
                                nc.tensor.matmul(
                                    psv[:], xs_hi[:, sl, tsl], wv_sbh[:, sl, :],
                                    start=(p == 0), stop=False, perf_mode=DR,
                                )
                                nc.tensor.matmul(
                                    psv[:], xs_hi[:, sl, tsl], wv_sbl[:, sl, :],
                                    start=False, stop=False, perf_mode=DR,
                                )
                                nc.tensor.matmul(
                                    psv[:], xs_lo[:, sl, tsl], wv_sbh[:, sl, :],
                                    start=False, stop=(p == KC // 2 - 1),
                                    perf_mode=DR,
                                )
                            nc.scalar.activation(
                                v_sb[:, tt, :], psv[:], AF.Copy, scale=1.0 / WS
                            )

                # ---- heads 0..6: attention + next head's projection ----
                for h in range(HQ - 1):
                    agen = a_stream(h + 1, psA)

                    def pop(kt, agen=agen):
                        next(agen, None)
                        if kt >= 10:
                            next(agen, None)

                    for qj in range(NQ):
                        ps_y, ps_d = emit_qj(h, qj, pop)
                        finalize(h, qj, ps_y, ps_d)
                    for _ in agen:
                        pass

            # ---- head 7: attention + output projection interleaved ----
            with (
                tc.tile_pool(name="wpc", bufs=6) as wpc,
                tc.tile_pool(name="obp", bufs=3) as obp,
                tc.tile_pool(name="psO", bufs=2, space="PSUM") as psO,
            ):
                def load_wp(fm):
                    wc = wpc.tile([128, 2, HQ, 128], FP8, name="wc")
                    nc.sync.dma_start(
                        wc[:], wp_hl[fm].rearrange("p (l h c) -> p l h c", l=2, h=HQ)
                    )
                    return wc

                def c_stream(n):
                    """Output projection for token strip n (16 feature tiles)."""
                    nsl = bass.ts(n, 512)
                    wcs = [load_wp(0), load_wp(1), load_wp(2)]
                    for fm in range(FM):
                        wc = wcs.pop(0)
                        yield
                        ps_o = psO.tile([128, 512], F32, name="pso")
                        for p in range(HQ // 2):
                            sl = slice(2 * p, 2 * p + 2)
                            nc.tensor.matmul(
                                ps_o[:], wc[:, 0, sl, :], yt_hi[:, sl, nsl],
                                start=(p == 0), stop=False, perf_mode=DR,
                            )
                            nc.tensor.matmul(
                                ps_o[:], wc[:, 0, sl, :], yt_lo[:, sl, nsl],
                                start=False, stop=False, perf_mode=DR,
                            )
                            nc.tensor.matmul(
                                ps_o[:], wc[:, 1, sl, :], yt_hi[:, sl, nsl],
                                start=False, stop=(p == HQ // 2 - 1),
                                perf_mode=DR,
                            )
                            if p == 0 and fm + 3 < FM:
                                wcs.append(load_wp(fm + 3))
                            yield
                        ob = obp.tile([128, 512], BF16, name="ob")
                        nc.scalar.activation(ob[:], ps_o[:], AF.Copy, scale=1.0 / WS)
                        nc.sync.dma_start(
                            outT[fm * 128 : (fm + 1) * 128, nsl], ob[:]
                        )
                        yield

                cgens = []

                _end = object()

                def pop7(kt):
                    for _ in range(7):
                        while cgens:
                            if next(cgens[0], _end) is _end:
                                cgens.pop(0)
                                continue
                            break

                for qj in range(NQ):
                    ps_y, ps_d = emit_qj(HQ - 1, qj, pop7)
                    flush_ones()
                    finalize(HQ - 1, qj, ps_y, ps_d)
                    cgens.append(c_stream(qj))
                # drain remaining output projection
                for g in cgens:
                    for _ in g:
                        pass

    nc.compile()
    return nc


def _get_nc():
    global _NC
    if _NC is None:
        _NC = build_nc()
    return _NC


def _split_hilo(a):
    """a = hi + lo with both parts e4m3."""
    hi = a.astype(E4)
    lo = (a - hi.astype(np.float32)).astype(E4)
    return hi, lo


def _prep_inputs(x, w_attn, w_proj):
    """Build the 8 per-core input maps from the full-problem arrays."""
    perm = np.concatenate([np.arange(0, HD, 2), np.arange(1, HD, 2)])

    f = np.arange(64, dtype=np.float64)
    inv = ROPE_THETA ** (-2.0 * f / HD)
    ang = inv[:, None] * np.arange(T, dtype=np.float64)[None, :]
    trigc = (np.cos(ang) / WS).astype(np.float32)
    trigs = (np.sin(ang) / WS).astype(np.float32)
    trigf = np.ascontiguousarray(np.concatenate([trigc, trigc], axis=0))
    trigw = np.ascontiguousarray(np.concatenate([trigs, trigs], axis=0))

    kk = np.arange(128)[:, None]
    qq = np.arange(128)[None, :]
    maskd = (kk <= qq).astype(BF)  # [128 k, 128 q] lower-triangle-valid

    w_attn = np.asarray(w_attn)
    w_proj = np.asarray(w_proj)
    x = np.asarray(x)

    in_maps = []
    for core in range(N_CORES):
        b, g = core // TP, core % TP
        # x features chunked: xhi[p, kc, t] = x[b].T[kc*128+p, t]
        xT = np.ascontiguousarray(x[b].T)  # [C, T] f32
        x_hi, x_lo = _split_hilo(xT)
        xhi = np.ascontiguousarray(x_hi.reshape(KC, 128, T).transpose(1, 0, 2))
        xlo = np.ascontiguousarray(x_lo.reshape(KC, 128, T).transpose(1, 0, 2))

        qrows = []
        for h in range(HQ):
            gh = g * HQ + h
            qrows.append(gh * HD + perm)
        for kv in range(HKV):
            gk = g * HKV + kv
            qrows.append(N_HEAD * HD + gk * HD + perm)
        qrows = np.concatenate(qrows)
        wqk = w_attn[qrows] * WS  # [1280, C] f32
        # stationary layout: wqk3[m, p, kc*128+col] = wqk[m*128+col, kc*128+p]
        wqk3 = np.ascontiguousarray(
            wqk.reshape(MQK, 128, KC, 128).transpose(0, 3, 2, 1).reshape(MQK, 128, C)
        )
        wqk3_hi, wqk3_lo = _split_hilo(wqk3)
        wqk_hl = np.ascontiguousarray(
            np.stack([wqk3_hi, wqk3_lo], axis=2).reshape(MQK, 128, 2 * C)
        )

        vrows = np.concatenate(
            [
                (N_HEAD + N_KV_HEAD) * HD + (g * HKV + kv) * HD + np.arange(HD)
                for kv in range(HKV)
            ]
        )
        wv = w_attn[vrows] * WS  # [256, C]
        # wv3[p, kc*256+c] = wv[c, kc*128+p]
        wv3 = np.ascontiguousarray(
            wv.reshape(HKV * HD, KC, 128).transpose(2, 1, 0).reshape(128, KC * HKV * HD)
        )
        wv3_hi, wv3_lo = _split_hilo(wv3)

        cols = np.arange(g * HQ * HD, (g + 1) * HQ * HD)
        wpg = w_proj[:, cols] * WS  # [C, 1024], rows = out features
        # wp5[fm, d, h*128+p] = wpg[fm*128+p, h*128+d]
        wp5 = np.ascontiguousarray(
            wpg.T.reshape(HQ, 128, FM, 128).transpose(2, 1, 0, 3).reshape(FM, 128, HQ * 128)
        )
        wp5_hi, wp5_lo = _split_hilo(wp5)
        wp_hl = np.ascontiguousarray(
            np.stack([wp5_hi, wp5_lo], axis=2).reshape(FM, 128, 2 * HQ * 128)
        )

        in_maps.append(
            {
                "xhi": xhi,
                "xlo": xlo,
                "wqk_hl": wqk_hl,
                "wv_hi": wv3_hi,
                "wv_lo": wv3_lo,
                "wp_hl": wp_hl,
                "trigf": trigf,
                "trigw": trigw,
                "maskd": maskd,
            }
        )
    return in_maps


def kernel(x, w_attn, w_proj):
    global LAST_RUN
    nc = _get_nc()
    in_maps = _prep_inputs(x, w_attn, w_proj)
    res = run_bass_kernel_spmd(nc, in_maps, core_ids=list(range(N_CORES)))
    LAST_RUN = res
    out = np.empty((B, T, C), dtype=np.float32)
    for b in range(B):
        acc = (
            res.results[TP * b]["outT"].astype(np.float32)
            + res.results[TP * b + 1]["outT"].astype(np.float32)
        )
        out[b] = acc.T
    return out


# revision 34
# speedup vs baseline: 1.2296x; 1.0100x over previous
"""Causal self-attention (GQA + RoPE) Trainium2 kernel, 8-way sharded.

Sharding: DP=4 over batch x TP=2 over kv-head groups (2 kv heads + their
8 q heads per group).  Each core computes its batch's qkv projection for
its head group, causal attention, and a partial c_proj (columns of
w_proj for its head group).  Host sums the two partial c_proj outputs
per batch.

Everything on-chip runs transposed ([feature, token] layout) so matmuls
contract along partitions; host transposes inputs/outputs.

Projection matmuls (qkv, v, c_proj) run as fp8e4 DoubleRow "triple-MMs":
each operand A is sent as A_hi + A_lo (both e4m3; hi = rounded value,
lo = rounded residual), and each pair of 128-deep contraction chunks is
computed with three DoubleRow matmuls
    hi*hi + hi*lo + lo*hi          (lo*lo ~ 0.06% -- dropped)
at half-rate each, i.e. 0.75x the bf16 cost with ~bf16 accuracy.
Weights are prescaled by 64 so their sigma ~ 1/45 lands in e4m3 normal
range; the inverse scale is folded into the RoPE trig tables / the
PSUM->SBUF copies.  Attention itself (QK, AV, softmax) stays bf16.

Pipeline: the attention inner loop leaves PE slack while ACT churns
exps, so the q/k projection work for head h+1 is interleaved into the
PE stream of head h's attention; head 7 interleaves c_proj instead.
The AV/denominator matmuls for tile kt are emitted two k-tiles late
(lag-2 software pipeline) so the exp -> mask chain never stalls the PE.
Causal masking only touches the one 128x128 triangle block per diagonal
tile; the valid column range of a diagonal tile is computed mask-free.

RoPE: w_attn q/k rows are permuted per-head to [even dims; odd dims] so
rotation pairs land at partition f and f+64 of the qkv psum tile:
  P  = ps * [c; c] (SBUF),  P2 = ps * [s; s] (PSUM)
  out[0:64]   = P[0:64]  - P2[64:128]
  out[64:128] = P2[0:64] + P[64:128]
(each combine reads one SBUF + one PSUM operand, which may sit at
different base partitions; two SBUF operands may not).

Softmax: att^T tiles ([k, q] layout) are exp'd on ACT without
max-subtraction (logits are O(6), fp32-safe).  Denominators: groups of
4 e-tiles are tree-summed on DVE and hit with one ones-MATRIX matmul
per group (deferred into the next group's PE stream) whose [128, 512]
output IS the denominator broadcast down all partitions -- the per-q
reciprocal is then a single DVE op, no extra broadcast needed.
"""

import math

import numpy as np
import ml_dtypes

import concourse.bass as bass
import concourse.mybir as mybir
import concourse.tile as tile
from concourse import bacc
from concourse.bass_utils import run_bass_kernel_spmd

ALU = mybir.AluOpType
AF = mybir.ActivationFunctionType
F32 = mybir.dt.float32
F32R = mybir.dt.float32r
BF16 = mybir.dt.bfloat16
FP8 = mybir.dt.float8e4
DR = mybir.MatmulPerfMode.DoubleRow
BF = ml_dtypes.bfloat16
E4 = ml_dtypes.float8_e4m3

# problem shape (hardcoded per contest rules)
B, T, C = 4, 2048, 2048
N_HEAD, N_KV_HEAD, HD = 16, 4, 128
ROPE_THETA = 10000.0

TP = 2            # head-group shards
DP = 4            # batch shards
HQ = N_HEAD // TP         # 8 q heads per core
HKV = N_KV_HEAD // TP     # 2 kv heads per core
NREP = N_HEAD // N_KV_HEAD  # 4
QK_ROWS = (HQ + HKV) * HD   # 1280
KC = C // 128     # 16 contraction tiles
NQ = T // 512     # 4 token strips
MQK = QK_ROWS // 128  # 10 feature tiles (8 q heads + 2 kv heads)
FM = C // 128     # 16 output feature tiles
SCALE = 1.0 / math.sqrt(HD)
WS = 64.0         # weight prescale for e4m3 range

N_CORES = 8

_NC = None        # cached compiled Bass module
LAST_RUN = None   # BassKernelResults of the most recent kernel() call


def build_nc():
    nc = bacc.Bacc(None, target_bir_lowering=False, debug=False)

    xhi = nc.declare_dram_parameter("xhi", [128, KC, T], FP8, isOutput=False)
    xlo = nc.declare_dram_parameter("xlo", [128, KC, T], FP8, isOutput=False)
    wqk_hl = nc.declare_dram_parameter("wqk_hl", [MQK, 128, 2 * KC * 128], FP8, isOutput=False)
    wv_hi = nc.declare_dram_parameter("wv_hi", [128, KC * HKV * HD], FP8, isOutput=False)
    wv_lo = nc.declare_dram_parameter("wv_lo", [128, KC * HKV * HD], FP8, isOutput=False)
    wp_hl = nc.declare_dram_parameter("wp_hl", [FM, 128, 2 * HQ * 128], FP8, isOutput=False)
    trigf = nc.declare_dram_parameter("trigf", [128, T], F32, isOutput=False)  # [c;c]/WS
    trigw = nc.declare_dram_parameter("trigw", [128, T], F32, isOutput=False)  # [s;s]/WS
    maskd = nc.declare_dram_parameter("maskd", [128, 128], BF16, isOutput=False)
    outT = nc.declare_dram_parameter("outT", [C, T], BF16, isOutput=True)

    with tile.TileContext(nc) as tc:
        with (
            tc.tile_pool(name="const", bufs=1) as const,
            tc.tile_pool(name="persist", bufs=1) as persist,
            tc.tile_pool(name="eb", bufs=10) as eb,
            tc.tile_pool(name="gag", bufs=2) as gag,
            tc.tile_pool(name="rb", bufs=1) as rb,
            tc.tile_pool(name="ytp", bufs=3) as ytp,
            tc.tile_pool(name="psS", bufs=3, space="PSUM") as psS,
            tc.tile_pool(name="psY", bufs=2, space="PSUM") as psY,
            tc.tile_pool(name="psD", bufs=1, space="PSUM") as psD,
        ):
            trigf_sb = const.tile([128, T], F32, name="trigf")
            trigw_sb = const.tile([128, T], F32, name="trigw")
            mask_sb = const.tile([128, 128], BF16, name="mask")
            ones_mat = const.tile([128, 128], BF16, name="onem")

            qrot = [persist.tile([128, T], BF16, name=f"qrot{h}") for h in range(HQ)]
            krot = [persist.tile([128, T], BF16, name=f"krot{h}") for h in range(HKV)]
            v_sb = persist.tile([128, T // 128, HKV * HD], BF16, name="vtok")
            yt_hi = persist.tile([128, HQ, T], FP8, name="ythi")
            yt_lo = persist.tile([128, HQ, T], FP8, name="ytlo")

            state = {"pending": None, "pending_ones": None}

            def finalize(h, qj, ps_y, ps_d):
                qsl = bass.ts(qj, 512)
                r_sb = rb.tile([128, 512], F32, name="r")
                nc.vector.reciprocal(r_sb[:], ps_d[:])
                tmp = ytp.tile([128, 512], BF16, name="ytmp")
                nc.vector.tensor_tensor(tmp[:], ps_y[:], r_sb[:], ALU.mult)
                nc.gpsimd.tensor_copy(yt_hi[:, h, qsl], tmp[:])
                nc.gpsimd.tensor_tensor(
                    yt_lo[:, h, qsl], tmp[:], yt_hi[:, h, qsl], ALU.subtract
                )

            def flush_ones():
                if state["pending_ones"] is not None:
                    po, st, sp, pd = state["pending_ones"]
                    nc.tensor.matmul(pd[:], ones_mat[:], po[:], start=st, stop=sp)
                    state["pending_ones"] = None

            def strip_tiles(h, qj, res):
                """Generator: attention for (h, qj), yielding once per k-tile.

                AV/denominator matmuls run two k-tiles behind QK/exp (lag-2
                software pipeline).  The (ps_y, ps_d) accumulators are stored
                in res[qj] (finalized by the caller).
                """
                kvh = h // NREP
                vs = lambda kt: v_sb[:, kt, kvh * HD : (kvh + 1) * HD]
                ps_y = psY.tile([128, 512], F32, name="psy")
                ps_d = psD.tile([128, 512], F32, name="psd")
                res[qj] = (ps_y, ps_d)
                nkt = 4 * qj + 4
                g0 = ga = g2 = None
                hist = []

                def emit_av(e, d, lo, kt):
                    first = kt == 0
                    last = kt == nkt - 1
                    mid = lo + 128
                    if d >= 0 and mid < 512:
                        # valid columns first (no mask dependency), then the
                        # masked 128-wide triangle block
                        nc.tensor.matmul(
                            ps_y[:, mid:512], vs(kt), e[:, mid:512],
                            start=first, stop=False,
                        )
                        nc.tensor.matmul(
                            ps_y[:, lo:mid], vs(kt), e[:, lo:mid],
                            start=False, stop=last,
                        )
                    else:
                        nc.tensor.matmul(
                            ps_y[:, lo:512], vs(kt), e[:, lo:512],
                            start=first, stop=last,
                        )
                    if d >= 0:
                        if d == 0:
                            flush_ones()
                        nc.tensor.matmul(
                            ps_d[:, lo:512], ones_mat[:], e[:, lo:512],
                            start=(qj == 0 and kt == 0), stop=last,
                        )

                for kt in range(nkt):
                    d = kt - 4 * qj
                    # diagonal tile d has valid q-columns only in [128d, 512)
                    lo = 128 * d if d > 0 else 0
                    qlo = qj * 512 + lo
                    ps_s = psS.tile([128, 512], F32, name="pss")
                    nc.tensor.matmul(
                        ps_s[:, lo:512],
                        krot[kvh][:, kt * 128 : (kt + 1) * 128],
                        qrot[h][:, qlo : (qj + 1) * 512],
                        start=True,
                        stop=True,
                    )
                    e = eb.tile([128, 512], BF16, name="e")
                    nc.scalar.activation(
                        e[:, lo:512], ps_s[:, lo:512], AF.Exp, scale=SCALE
                    )
                    if d >= 0:
                        # mask only the 128x128 triangle block (on GPSIMD --
                        # the lag-2 AV emission gives the chain plenty of slack)
                        nc.gpsimd.tensor_tensor(
                            e[:, lo : lo + 128], e[:, lo : lo + 128],
                            mask_sb[:], ALU.mult,
                        )
                    else:
                        # full groups: tree-sum 4 e-tiles (first add on GPSIMD,
                        # rest on DVE), one deferred ones-matmul per group
                        # (emitted in a later PE slot so the PE never waits on
                        # the adds).
                        ph = kt % 4
                        if ph == 0:
                            g0 = e
                        elif ph == 1:
                            ga = gag.tile([128, 512], BF16, name="ga")
                            nc.gpsimd.tensor_tensor(ga[:], g0[:], e[:], ALU.add)
                        elif ph == 2:
                            g2 = e
                        else:
                            gs = gag.tile([128, 512], BF16, name="gs")
                            nc.vector.tensor_tensor(gs[:], g2[:], e[:], ALU.add)
                            nc.vector.tensor_tensor(gs[:], gs[:], ga[:], ALU.add)
                            flush_ones()
                            grp = kt // 4
                            state["pending_ones"] = (gs, grp == 0, False, ps_d)
                    hist.append((e, d, lo, kt))
                    if len(hist) > 2:
                        emit_av(*hist.pop(0))
                    yield
                for item in hist:
                    emit_av(*item)

            def emit_qj(h, qj, pop):
                """Attention for (h, qj), with pop() called once per k-tile."""
                res = {}
                gen = strip_tiles(h, qj, res)
                kt = 0
                while next(gen, _END) is not _END:
                    pop(kt)
                    kt += 1
                return res[qj]

            _END = object()

            # ======== projection machinery (lives through heads 0..6) ========
            with (
                tc.tile_pool(name="xa", bufs=1) as xa,
                tc.tile_pool(name="wm", bufs=4) as wm,
                tc.tile_pool(name="ta", bufs=1) as ta,
                tc.tile_pool(name="psA", bufs=1, space="PSUM") as psA,
                tc.tile_pool(name="psP2", bufs=1, space="PSUM") as psP2,
            ):
                xs_hi = xa.tile([128, KC, T], FP8, name="xshi")
                xs_lo = xa.tile([128, KC, T], FP8, name="xslo")

                def load_wm(m):
                    w = wm.tile([128, 2, KC, 128], FP8, name="wmhl")
                    nc.sync.dma_start(
                        w[:], wqk_hl[m].rearrange("p (l kc c) -> p l kc c", l=2, kc=KC)
                    )
                    return w

                def proj_mms(ps, w, nsl):
                    """Triple-MM qkv projection chunk stream for one strip."""
                    for p in range(KC // 2):
                        sl = slice(2 * p, 2 * p + 2)
                        nc.tensor.matmul(
                            ps[:], w[:, 0, sl, :], xs_hi[:, sl, nsl],
                            start=(p == 0), stop=False, perf_mode=DR,
                        )
                        nc.tensor.matmul(
                            ps[:], w[:, 0, sl, :], xs_lo[:, sl, nsl],
                            start=False, stop=False, perf_mode=DR,
                        )
                        nc.tensor.matmul(
                            ps[:], w[:, 1, sl, :], xs_hi[:, sl, nsl],
                            start=False, stop=(p == KC // 2 - 1), perf_mode=DR,
                        )
                        yield

                def rope_ops(m, n, ps):
                    """The four RoPE ops for one (feature tile, strip) pair."""
                    dst = qrot[m] if m < HQ else krot[m - HQ]
                    nsl = bass.ts(n, 512)
                    pt = ta.tile([128, 512], F32, name="pt")
                    p2 = psP2.tile([128, 512], F32, name="p2")
                    yield nc.vector.tensor_tensor(
                        pt[:], ps[:], trigf_sb[:, nsl], ALU.mult
                    )
                    yield nc.vector.tensor_tensor(
                        p2[:], ps[:], trigw_sb[:, nsl], ALU.mult
                    )
                    yield nc.vector.tensor_tensor(
                        dst[0:64, nsl], pt[0:64, :], p2[64:128, :], ALU.subtract
                    )
                    yield nc.vector.tensor_tensor(
                        dst[64:128, nsl], p2[0:64, :], pt[64:128, :], ALU.add
                    )

                def a_stream(m, pool):
                    w = load_wm(m)
                    yield
                    for n in range(NQ):
                        nsl = bass.ts(n, 512)
                        ps = pool.tile([128, 512], F32, name="psA")
                        yield from proj_mms(ps[:], w, nsl)
                        for _ in rope_ops(m, n, ps):
                            yield

                # ---- A0: v projection + k heads + q head 0 (pure PE phase) ----
                with tc.tile_pool(name="wvp", bufs=1) as wvp:
                    wv_sbh = wvp.tile([128, KC, HKV * HD], FP8, name="wvh")
                    wv_sbl = wvp.tile([128, KC, HKV * HD], FP8, name="wvl")
                    wk0 = load_wm(HQ)
                    for i in range(4):
                        ksl = slice(4 * i, 4 * i + 4)
                        nc.sync.dma_start(xs_hi[:, ksl, bass.ts(0, 512)],
                                          xhi[:, ksl, bass.ts(0, 512)])
                        nc.sync.dma_start(xs_lo[:, ksl, bass.ts(0, 512)],
                                          xlo[:, ksl, bass.ts(0, 512)])
                        if i == 0:
                            wk1 = load_wm(HQ + 1)
                        elif i == 1:
                            wq0 = load_wm(0)
                    nc.sync.dma_start(
                        wv_sbh[:], wv_hi.rearrange("p (kc c) -> p kc c", kc=KC)
                    )
                    nc.sync.dma_start(
                        wv_sbl[:], wv_lo.rearrange("p (kc c) -> p kc c", kc=KC)
                    )
                    nc.sync.dma_start(trigf_sb[:], trigf[:])
                    nc.sync.dma_start(trigw_sb[:], trigw[:])
                    nc.sync.dma_start(mask_sb[:], maskd[:])
                    nc.vector.memset(ones_mat[:], 1.0)
                    for n in range(NQ):
                        nsl = bass.ts(n, 512)
                        for m, w in ((HQ, wk0), (HQ + 1, wk1), (0, wq0)):
                            ps = psY.tile([128, 512], F32, name="psy")
                            for _ in proj_mms(ps[:], w, nsl):
                                pass
                            for _ in rope_ops(m, n, ps):
                                pass
                        if n + 1 < NQ:
                            nsl_next = bass.ts(n + 1, 512)
                            for i in range(4):
                                ksl = slice(4 * i, 4 * i + 4)
                                nc.sync.dma_start(xs_hi[:, ksl, nsl_next],
                                                  xhi[:, ksl, nsl_next])
                                nc.sync.dma_start(xs_lo[:, ksl, nsl_next],
                                                  xlo[:, ksl, nsl_next])
                        for tt in range(4 * n, 4 * n + 4):
                            # reuse the attention-phase psum slots during A0
                            tsl = slice(tt * 128, (tt + 1) * 128)
                            psv = psS.tile([128, 512], F32, name="pss")[
                                :, : HKV * HD
                            ]
                            for p in range(KC // 2):
                                sl = slice(2 * p, 2 * p + 2)
                                nc.tensor.matmul(
                                    psv[:], xs_hi[:, sl, tsl], wv_sbh[:, sl, :],
                                    start=(p == 0), stop=False, perf_mode=DR,
                                )
                                nc.tensor.matmul(
                                    psv[:], xs_hi[:, sl, tsl], wv_sbl[:, sl, :],
                                    start=False, stop=False, perf_mode=DR,
                                )
                                nc.tensor.matmul(
                                    psv[:], xs_lo[:, sl, tsl], wv_sbh[:, sl, :],
                                    start=False, stop=(p == KC // 2 - 1),
                                    perf_mode=DR,
                                )
                            nc.scalar.activation(
                                v_sb[:, tt, :], psv[:], AF.Copy, scale=1.0 / WS
                            )

                # ---- heads 0..6: attention + next head's projection ----
                for h in range(HQ - 1):
                    agen = a_stream(h + 1, psA)

                    def pop(kt, agen=agen):
                        next(agen, None)
                        if kt >= 10:
                            next(agen, None)

                    for qj in range(NQ):
                        ps_y, ps_d = emit_qj(h, qj, pop)
                        finalize(h, qj, ps_y, ps_d)
                    for _ in agen:
                        pass

            # ---- head 7: attention + output projection interleaved ----
            with (
                tc.tile_pool(name="wpc", bufs=6) as wpc,
                tc.tile_pool(name="obp", bufs=3) as obp,
                tc.tile_pool(name="psO", bufs=2, space="PSUM") as psO,
            ):
                def load_wp(fm):
                    wc = wpc.tile([128, 2, HQ, 128], FP8, name="wc")
                    nc.sync.dma_start(
                        wc[:], wp_hl[fm].rearrange("p (l h c) -> p l h c", l=2, h=HQ)
                    )
                    return wc

                def c_stream(n):
                    """Output projection for token strip n (16 feature tiles)."""
                    nsl = bass.ts(n, 512)
                    wcs = [load_wp(0), load_wp(1), load_wp(2)]
                    for fm in range(FM):
                        wc = wcs.pop(0)
                        yield
                        ps_o = psO.tile([128, 512], F32, name="pso")
                        for p in range(HQ // 2):
                            sl = slice(2 * p, 2 * p + 2)
                            nc.tensor.matmul(
                                ps_o[:], wc[:, 0, sl, :], yt_hi[:, sl, nsl],
                                start=(p == 0), stop=False, perf_mode=DR,
                            )
                            nc.tensor.matmul(
                                ps_o[:], wc[:, 0, sl, :], yt_lo[:, sl, nsl],
                                start=False, stop=False, perf_mode=DR,
                            )
                            nc.tensor.matmul(
                                ps_o[:], wc[:, 1, sl, :], yt_hi[:, sl, nsl],
                                start=False, stop=(p == HQ // 2 - 1),
                                perf_mode=DR,
                            )
                            if p == 0 and fm + 3 < FM:
                                wcs.append(load_wp(fm + 3))
                            yield
                        ob = obp.tile([128, 512], BF16, name="ob")
                        nc.scalar.activation(ob[:], ps_o[:], AF.Copy, scale=1.0 / WS)
                        nc.sync.dma_start(
                            outT[fm * 128 : (fm + 1) * 128, nsl], ob[:]
                        )
                        yield

                cgens = []

                _end = object()

                def pop7(kt):
                    for _ in range(7):
                        while cgens:
                            if next(cgens[0], _end) is _end:
                                cgens.pop(0)
                                continue
                            break

                for qj in range(NQ):
                    ps_y, ps_d = emit_qj(HQ - 1, qj, pop7)
                    flush_ones()
                    finalize(HQ - 1, qj, ps_y, ps_d)
                    cgens.append(c_stream(qj))
                # drain remaining output projection
                for g in cgens:
                    for _ in g:
                        pass

    nc.compile()
    return nc


def _get_nc():
    global _NC
    if _NC is None:
        _NC = build_nc()
    return _NC


def _split_hilo(a):
    """a = hi + lo with both parts e4m3."""
    hi = a.astype(E4)
    lo = (a - hi.astype(np.float32)).astype(E4)
    return hi, lo


def _prep_inputs(x, w_attn, w_proj):
    """Build the 8 per-core input maps from the full-problem arrays."""
    perm = np.concatenate([np.arange(0, HD, 2), np.arange(1, HD, 2)])

    f = np.arange(64, dtype=np.float64)
    inv = ROPE_THETA ** (-2.0 * f / HD)
    ang = inv[:, None] * np.arange(T, dtype=np.float64)[None, :]
    trigc = (np.cos(ang) / WS).astype(np.float32)
    trigs = (np.sin(ang) / WS).astype(np.float32)
    trigf = np.ascontiguousarray(np.concatenate([trigc, trigc], axis=0))
    trigw = np.ascontiguousarray(np.concatenate([trigs, trigs], axis=0))

    kk = np.arange(128)[:, None]
    qq = np.arange(128)[None, :]
    maskd = (kk <= qq).astype(BF)  # [128 k, 128 q] lower-triangle-valid

    w_attn = np.asarray(w_attn)
    w_proj = np.asarray(w_proj)
    x = np.asarray(x)

    in_maps = []
    for core in range(N_CORES):
        b, g = core // TP, core % TP
        # x features chunked: xhi[p, kc, t] = x[b].T[kc*128+p, t]
        xT = np.ascontiguousarray(x[b].T)  # [C, T] f32
        x_hi, x_lo = _split_hilo(xT)
        xhi = np.ascontiguousarray(x_hi.reshape(KC, 128, T).transpose(1, 0, 2))
        xlo = np.ascontiguousarray(x_lo.reshape(KC, 128, T).transpose(1, 0, 2))

        qrows = []
        for h in range(HQ):
            gh = g * HQ + h
            qrows.append(gh * HD + perm)
        for kv in range(HKV):
            gk = g * HKV + kv
            qrows.append(N_HEAD * HD + gk * HD + perm)
        qrows = np.concatenate(qrows)
        wqk = w_attn[qrows] * WS  # [1280, C] f32
        # stationary layout: wqk3[m, p, kc*128+col] = wqk[m*128+col, kc*128+p]
        wqk3 = np.ascontiguousarray(
            wqk.reshape(MQK, 128, KC, 128).transpose(0, 3, 2, 1).reshape(MQK, 128, C)
        )
        wqk3_hi, wqk3_lo = _split_hilo(wqk3)
        wqk_hl = np.ascontiguousarray(
            np.stack([wqk3_hi, wqk3_lo], axis=2).reshape(MQK, 128, 2 * C)
        )

        vrows = np.concatenate(
            [
                (N_HEAD + N_KV_HEAD) * HD + (g * HKV + kv) * HD + np.arange(HD)
                for kv in range(HKV)
            ]
        )
        wv = w_attn[vrows] * WS  # [256, C]
        # wv3[p, kc*256+c] = wv[c, kc*128+p]
        wv3 = np.ascontiguousarray(
            wv.reshape(HKV * HD, KC, 128).transpose(2, 1, 0).reshape(128, KC * HKV * HD)
        )
        wv3_hi, wv3_lo = _split_hilo(wv3)

        cols = np.arange(g * HQ * HD, (g + 1) * HQ * HD)
        wpg = w_proj[:, cols] * WS  # [C, 1024], rows = out features
        # wp5[fm, d, h*128+p] = wpg[fm*128+p, h*128+d]
        wp5 = np.ascontiguousarray(
            wpg.T.reshape(HQ, 128, FM, 128).transpose(2, 1, 0, 3).reshape(FM, 128, HQ * 128)
        )
        wp5_hi, wp5_lo = _split_hilo(wp5)
        wp_hl = np.ascontiguousarray(
            np.stack([wp5_hi, wp5_lo], axis=2).reshape(FM, 128, 2 * HQ * 128)
        )

        in_maps.append(
            {
                "xhi": xhi,
                "xlo": xlo,
                "wqk_hl": wqk_hl,
                "wv_hi": wv3_hi,
                "wv_lo": wv3_lo,
                "wp_hl": wp_hl,
                "trigf": trigf,
                "trigw": trigw,
                "maskd": maskd,
            }
        )
    return in_maps


def kernel(x, w_attn, w_proj):
    global LAST_RUN
    nc = _get_nc()
    in_maps = _prep_inputs(x, w_attn, w_proj)
    res = run_bass_kernel_spmd(nc, in_maps, core_ids=list(range(N_CORES)))
    LAST_RUN = res
    out = np.empty((B, T, C), dtype=np.float32)
    for b in range(B):
        acc = (
            res.results[TP * b]["outT"].astype(np.float32)
            + res.results[TP * b + 1]["outT"].astype(np.float32)
        )
        out[b] = acc.T
    return out
